# revision 76
# baseline (speedup 1.0000x reference)
"""CRF integration (nn_CRFIntegrationModule) Trainium2 kernel.

One image per NeuronCore (B=8 -> 8 cores).  Each direction's 32-step windowed
scan is one hardware tensor_tensor_scan on a pre-corrected input (windowed
linear recurrence):

    A32[n] = (A32[n-1] + u'[n-1]) * m[n-1]
    u'[k]  = u[k] - M[k] * u[k -/+ 32]        (M = windowed mask product)

The per-path exponent exp(sum plog) is factored out through the global plog
prefix P (one scan):  awd = e^{P} * scan(u * e^{-P}, m), so every scan
transition is the exact mask and no exp factors compound along paths.
M comes from a mask cumsum scan + windowed difference + ==32 compare, all on
GPSIMD (one engine-local chain).

Data flow: the H phase loads every input plane once (SWDGE cast loads to
fp16, multi-plane fused calls, all enqueued up front in readiness order —
engine queues are in-order FIFOs, so issue order is chosen to match expected
operand arrival) into persistent row-major SBUF planes and runs the
horizontal scans.  The V phase reads its inputs from those SBUF planes
through the XBAR DMA transpose (issued on the otherwise idle SP sequencer:
no compute engine spends time staging or transposing), scans vertically, and
PE-transposes the results back, accumulating into the H totals in place.
Work is split across DVE / GPSIMD / Act so every stream stays busy: the
mask-window machinery and the aw (weight) scans run on GPSIMD, deferred
until their inputs are ready so they never head-block the queue; the final
blend divides on GPSIMD (no Ln/Exp activation-table swaps anywhere).
"""
import os
import sys

for _p in ("/opt/trn_rl_repo", "/root/.axon_site/_ro/trn_rl_repo"):
    if os.path.isdir(_p) and _p not in sys.path:
        sys.path.insert(0, _p)
        break

import numpy as np
import concourse.bacc as bacc
import concourse.mybir as mybir
import concourse.tile as tile
from concourse import masks
from concourse.bass_utils import run_bass_kernel_spmd

Alu = mybir.AluOpType
ActF = mybir.ActivationFunctionType
F32 = mybir.dt.float32
I32 = mybir.dt.int32
BF16 = mybir.dt.bfloat16
F16 = mybir.dt.float16

B, H, W = 8, 352, 1216
R = 32          # MAXRANGE
CLIP = 5.0      # CLIPVARIANCE
EM5 = float(np.exp(-CLIP))
PAD = 32

# H-phase geometry: row segments (partitions = rows), padded row-major planes
RSEGS = [(0, 128), (128, 128), (256, 96)]          # (row0, height)
FH = PAD + 1280                                    # transposes read 1280 wide
HI = PAD + W
FHN = HI + 2                                       # narrow scratch width
# V-phase geometry: transposed layout, 2 chunks x 5 col-segments of <=128 cols
VSEG = H + PAD                                     # 384 per col-seg span
NCS = 5
FV = PAD + NCS * VSEG + 8                          # 1960
VHI = PAD + (NCS - 1) * VSEG + H                   # 1920
FVB = PAD + 2 * NCS * VSEG + 8                     # 3880: both chunks
VCHUNKS = [(0, 640), (640, 576)]


def _pad_memsets(nc, t, lo, hi, F, vgaps=False, eng=None):
    """Zero the pad strips of a [128, F] tile (head, tail, V inter-seg gaps)."""
    g_ = eng if eng is not None else nc.vector
    g_.memset(t[:, 0:lo], 0.0)
    g_.memset(t[:, hi:F], 0.0)
    if vgaps:
        g = t[:, PAD:PAD + 5 * VSEG].rearrange("p (s c) -> p s c", s=5)
        g_.memset(g[:, :, H:VSEG], 0.0)


def _mw_chain(nc, m, Mw, Sm, cs, lo, hi, sub_eng=None):
    """Windowed mask product M[n] = prod_{j=1..32} m[n-j]: cumsum scan (DVE:
    hardware scans only exist there) + windowed difference + ==32 compare."""
    g = nc.gpsimd
    sub = sub_eng if sub_eng is not None else g
    nc.vector.tensor_tensor_scan(
        Sm[:, 1:hi + 33], m[:, 0:hi + 32], m[:, 0:hi + 32],
        0.0, op0=Alu.add, op1=Alu.bypass)
    sub.tensor_tensor(cs[:, lo:hi + 33], Sm[:, lo:hi + 33],
                      Sm[:, lo - R:hi + 33 - R], op=Alu.subtract)
    g.tensor_scalar(Mw[:, lo:hi + 33], cs[:, lo:hi + 33], float(R) - 0.5,
                    None, op0=Alu.is_ge)


def _axis_prefix(nc, p, Pp, eN, eP, lo, hi):
    """Plog prefix scan + its exponentials: first DVE/Act work of a segment
    (issued before the E-plane exp/clamp so it never waits on them)."""
    nc.vector.tensor_tensor_scan(
        Pp[:, lo:hi], p[:, lo - 1:hi - 1], p[:, lo - 1:hi - 1], 0.0,
        op0=Alu.add, op1=Alu.bypass)
    nc.scalar.activation(eN[:, lo:hi], Pp[:, lo:hi], ActF.Exp, scale=-1.0)
    nc.scalar.activation(eP[:, lo:hi], Pp[:, lo:hi], ActF.Exp)


def _axis_main(nc, m, p, E0, E1, D, Dh, u0, u1, Mw, c1, c2,
               Pp, eN, eP, u0p, u1p, E0p, E1p, lo, hi, awd_out,
               esub_eng=None):
    """DVE stream of one axis pair: u-hat, windowed pre-corrections, the two
    awd scans, and the awd output.  The aw (weight) scans BL/BR run on GPSIMD
    and are emitted separately via _axis_aw_scans; prefix via _axis_prefix;
    scan outputs alias u0/u1/E0/E1."""
    v = nc.vector
    sc = v.tensor_tensor_scan
    # u-hat = E * D * e^{-P}
    v.tensor_mul(Dh[:, lo:hi], D[:, lo:hi], eN[:, lo:hi])
    v.tensor_mul(u0[:, lo:hi], E0[:, lo:hi], Dh[:, lo:hi])
    v.tensor_mul(u1[:, lo:hi], E1[:, lo:hi], Dh[:, lo:hi])
    # windowed pre-corrections  u'[m] = u[m] - M * u[m -/+ 32]  (mask-only)
    v.tensor_mul(c1[:, lo:hi], Mw[:, lo:hi], u0[:, lo - R:hi - R])
    v.tensor_sub(u0p[:, lo:hi], u0[:, lo:hi], c1[:, lo:hi])
    v.tensor_mul(c2[:, lo:hi], Mw[:, lo + R + 1:hi + R + 1],
                 u1[:, lo + R:hi + R])
    v.tensor_sub(u1p[:, lo:hi], u1[:, lo:hi], c2[:, lo:hi])
    es = esub_eng if esub_eng is not None else v
    v.tensor_mul(c1[:, lo:hi], Mw[:, lo:hi], E0[:, lo - R:hi - R])
    es.tensor_tensor(E0p[:, lo:hi], E0[:, lo:hi], c1[:, lo:hi],
                     op=Alu.subtract)
    v.tensor_mul(c2[:, lo:hi], Mw[:, lo + R + 1:hi + R + 1],
                 E1[:, lo + R:hi + R])
    es.tensor_tensor(E1p[:, lo:hi], E1[:, lo:hi], c2[:, lo:hi],
                     op=Alu.subtract)
    AL, AR = u0, u1
    sc(AL[:, lo:hi], u0p[:, lo - 1:hi - 1], m[:, lo - 1:hi - 1], 0.0,
       op0=Alu.add, op1=Alu.mult)
    sc(AR[:, lo:hi][:, ::-1], u1p[:, lo + 1:hi + 1][:, ::-1],
       m[:, lo + 1:hi + 1][:, ::-1], 0.0, op0=Alu.add, op1=Alu.mult)
    v.tensor_add(c1[:, lo:hi], AL[:, lo:hi], AR[:, lo:hi])
    v.tensor_mul(awd_out, c1[:, lo:hi], eP[:, lo:hi])


def _axis_aw_scans(nc, m, E0p, E1p, BL, BR, lo, hi, eng=None):
    """Deferred weight scans (aw numerator); DVE (scans are DVE-only)."""
    g = nc.vector
    g.tensor_tensor_scan(BL[:, lo:hi], E0p[:, lo - 1:hi - 1],
                         m[:, lo - 1:hi - 1], 0.0, op0=Alu.add, op1=Alu.mult)
    g.tensor_tensor_scan(BR[:, lo:hi][:, ::-1], E1p[:, lo + 1:hi + 1][:, ::-1],
                         m[:, lo + 1:hi + 1][:, ::-1], 0.0,
                         op0=Alu.add, op1=Alu.mult)


def build_program():
    nc = bacc.Bacc("TRN2", target_bir_lowering=False, debug=False, dynamic_dma_scratch_size=16384)

    pred_log = nc.dram_tensor("pred_log", [2, H, W], F32, kind="ExternalInput").ap()
    mask = nc.dram_tensor("mask", [1, H, W], I32, kind="ExternalInput").ap()
    variance = nc.dram_tensor("variance", [4, H, W], F32, kind="ExternalInput").ap()
    depth_cur = nc.dram_tensor("depth_cur", [1, H, W], F32, kind="ExternalInput").ap()
    depth_orig = nc.dram_tensor("depth_orig", [1, H, W], F32, kind="ExternalInput").ap()
    lam = nc.dram_tensor("lam", [1], F32, kind="ExternalInput").ap()
    depthout = nc.dram_tensor("depthout", [1, H, W], F32, kind="ExternalOutput").ap()

    g = nc.gpsimd
    lo = PAD
    with tile.TileContext(nc, pool_alloc_mode="queue") as tc:
        with tc.tile_pool(name="const", bufs=1) as cp, \
             tc.tile_pool(name="persist", bufs=1) as ps, \
             tc.tile_pool(name="psum", bufs=8, space="PSUM") as pp:
            ident = cp.tile([128, 128], F16, tag="ident")
            masks.make_identity(nc, ident[:])
            # natural_log_exp_and_others: covers every activation we use, so
            # the act-table pass never needs to insert mid-kernel swaps
            nc.scalar.add_instruction(mybir.InstLoadActFuncSet(
                name=nc.get_next_instruction_name(), act_func_set_id=6,
                ins=[], outs=[]))
            # band matrices: the V-direction 32-row window sum is a PE
            # convolution over rows in the row-major layout
            # W1[k, i] = 1 iff i-32 <= k < i (own segment rows)
            W1 = cp.tile([128, 128], F16, tag="W1")
            nc.gpsimd.memset(W1[:, :], 1.0)
            nc.gpsimd.affine_select(W1[:, :], W1[:, :], base=32,
                                    channel_multiplier=1,
                                    pattern=[[-1, 128]],
                                    compare_op=Alu.is_ge, fill=0.0)
            nc.gpsimd.affine_select(W1[:, :], W1[:, :], base=-1,
                                    channel_multiplier=-1,
                                    pattern=[[1, 128]],
                                    compare_op=Alu.is_ge, fill=0.0)
            # W2[k, i] = 1 iff k >= 96 + i (previous segment's tail rows)
            W2 = cp.tile([128, 128], F16, tag="W2")
            nc.gpsimd.memset(W2[:, :], 1.0)
            nc.gpsimd.affine_select(W2[:, :], W2[:, :], base=-96,
                                    channel_multiplier=1,
                                    pattern=[[-1, 128]],
                                    compare_op=Alu.is_ge, fill=0.0)
            lam_t = cp.tile([128, 1], F32, tag="lam")

            # persistent row-major fp16 planes, 3 row-seg slots of width FH
            mS = ps.tile([128, 3 * FH], F16, tag="mS")
            p01S = ps.tile([128, 6 * FH], F16, tag="p01S")    # [seg][p0,p1]
            DS = ps.tile([128, 3 * FH], F16, tag="DS")
            E23S = ps.tile([128, 6 * FH], F16, tag="E23S")    # [seg][E2,E3]
            twH = ps.tile([128, 3 * W], F16, tag="twH")
            twdH = ps.tile([128, 3 * W], F16, tag="twdH")
            # V input planes (written by SP DMA transposes; pads and
            # inter-seg gaps zeroed in the prologue while DVE is idle).
            # One full-width tile per plane: col-groups 0-4 are V chunk 0,
            # 5-9 chunk 1; each chunk's compute uses a [128, FV] window.
            vin = {}
            for nm in ("vm", "vq", "vD", "vE0", "vE1"):
                vin[nm] = ps.tile([128, FVB], F16, tag=f"{nm}B",
                                  name=f"{nm}B")
            MwVH = ps.tile([128, 3 * FH], F16, tag="MwVH")
            vMwB = ps.tile([128, FVB], F16, tag="vMwB")

            # ---- prologue ------------------------------------------------
            # pad zeroing first (everything idle at t=0; loads write only
            # the interiors, but region tracking may be whole-tile, so pads
            # go first to keep the load DMAs unblocked)
            m3 = mS.rearrange("p (s c) -> p s c", s=3)
            p6 = p01S.rearrange("p (s c) -> p s c", s=6)
            D3 = DS.rearrange("p (s c) -> p s c", s=3)
            E6 = E23S.rearrange("p (s c) -> p s c", s=6)
            for t3 in (p6, E6):
                g.memset(t3[:, :, 0:PAD], 0.0)
                g.memset(t3[:, :, HI:FH], 0.0)
            for t3 in (m3, D3):
                nc.vector.memset(t3[:, :, 0:PAD], 0.0)
                nc.vector.memset(t3[:, :, HI:FH], 0.0)
            # seg2 slots hold 96 rows; full-partition ops read rows 96:127
            g.memset(p6[96:128, 4:6, PAD:HI], 0.0)
            nc.vector.memset(m3[96:128, 2:3, PAD:HI], 0.0)
            nc.vector.memset(D3[96:128, 2:3, PAD:HI], 0.0)
            for t in list(vin.values()) + [vMwB]:
                nc.vector.memset(t[:, 0:lo], 0.0)
                nc.vector.memset(t[:, PAD + 10 * VSEG:FVB], 0.0)
                gv = t[:, PAD:PAD + 10 * VSEG].rearrange(
                    "p (s c) -> p s c", s=10)
                nc.vector.memset(gv[:, :, H:VSEG], 0.0)
            mw3 = MwVH.rearrange("p (s c) -> p s c", s=3)
            nc.vector.memset(mw3[:, :, 0:PAD], 0.0)
            nc.vector.memset(mw3[:, :, HI:FH], 0.0)


            _h_phase(nc, tc, mask, pred_log, variance, depth_cur,
                     depth_orig, mS, p01S, DS, E23S, twH, twdH, m3, D3,
                     pp, W1, W2, MwVH)

            # V-input transposes on the idle SP sequencer.  The manual wait
            # overrides keep them out of the early DMA window (the framework
            # serializes the in-flight DMA stream, so transposes scheduled
            # between the critical H loads would push those loads out).
            early, late = _v_transposes(vin, mS, p01S, DS, E23S)
            with tc.tile_wait_until(0.036):
                for args in early:
                    nc.sync.dma_start_transpose(*args)
            with tc.tile_wait_until(0.044):
                for args in late:
                    nc.sync.dma_start_transpose(*args)

            with tc.tile_wait_until(0.044):
                for args in _vmw_transposes(MwVH, vMwB):
                    nc.sync.dma_start_transpose(*args)

            with tc.tile_wait_until(0.040):
                nc.sync.dma_start(lam_t[:, 0:1], lam.partition_broadcast(128))
            bl = dict(mS=mS, twH=twH, twdH=twdH, lam_t=lam_t,
                      depthout=depthout)
            _v_phase(nc, tc, pp, ident, depth_orig, vin, twH, twdH, vMwB, bl)
    nc.finalize()
    return nc


def _h_phase(nc, tc, mask, pred_log, variance, depth, depth_orig,
             mS, p01S, DS, E23S, twH, twdH, m3, D3, pp, W1, W2, MwVH):
    v = nc.vector
    g = nc.gpsimd
    with tc.tile_pool(name="hp", bufs=1) as hp:
        def t_(tag, w=FH, dt=F16, bufs=1):
            return hp.tile([128, w], dt, tag=tag, name=tag, bufs=bufs)

        c1, c2 = t_("c1", dt=BF16), t_("c2", dt=BF16)
        mwc = t_("mwc", dt=BF16)
        u0p, u1p = t_("u0p", dt=BF16), t_("u1p", dt=BF16)
        E0p = [t_(f"E0p{i}") for i in range(3)]
        E1p = [t_(f"E1p{i}") for i in range(3)]
        Mw = [t_(f"Mw{i}") for i in range(3)]
        Sm = t_("Sm")
        E01S = t_("E01S", 6 * FH)                      # [seg][E0,E1]
        for t in (u0p, u1p):
            _pad_memsets(nc, t, PAD, HI, FH)
        e6 = E01S.rearrange("p (s c) -> p s c", s=6)
        nc.vector.memset(e6[:, :, 0:PAD], 0.0)
        nc.vector.memset(e6[:, :, HI:FH], 0.0)
        nc.vector.memset(e6[96:128, 4:6, PAD:HI], 0.0)
        g.memset(Sm[:, 0:1], 0.0)

        # ---- load batch 1: seg0+1 essentials (free DMA-window slots) ----
        pq = p01S.rearrange("p (s c) -> p s c", s=6)
        E6q = E23S.rearrange("p (s c) -> p s c", s=6)
        g.dma_start(pq[:, 0:4:2, PAD:HI],
                    pred_log[0, 0:256, :].rearrange("(s p) c -> p s c", p=128))
        g.dma_start(m3[:, 0:2, PAD:HI],
                    mask[0, 0:256, :].rearrange("(s p) c -> p s c", p=128))
        g.dma_start(e6[:, 0:4:2, PAD:HI],
                    variance[0, 0:256, :].rearrange("(s p) c -> p s c", p=128))
        g.dma_start(e6[:, 1:4:2, PAD:HI],
                    variance[1, 0:256, :].rearrange("(s p) c -> p s c", p=128))
        g.dma_start(D3[:, 0:2, PAD:HI],
                    depth[0, 0:256, :].rearrange("(s p) c -> p s c", p=128))

        # seg0's mask-window chain slots between the two enqueue batches, so
        # the later enqueues' DMA-window waits are already satisfied when the
        # Pool engine reaches them (no head-of-line stall either way)
        _mw_chain(nc, mS[:, 0:FH], Mw[0], Sm, mwc, PAD, HI)

        # ---- load batch 2: seg2 + the V-phase E planes -------------------
        g.dma_start(mS[0:96, 2 * FH + PAD:2 * FH + HI], mask[0, 256:352, :])
        g.dma_start(
            p01S[0:96, 4 * FH:6 * FH].rearrange(
                "p (q c) -> p q c", q=2)[:, :, PAD:HI],
            pred_log[0:2, 256:352, :].rearrange("q r c -> r q c"))
        g.dma_start(DS[0:96, 2 * FH + PAD:2 * FH + HI],
                    depth[0, 256:352, :])
        g.dma_start(
            E01S[0:96, 4 * FH:6 * FH].rearrange(
                "p (q c) -> p q c", q=2)[:, :, PAD:HI],
            variance[0:2, 256:352, :].rearrange("q r c -> r q c"))
        g.dma_start(pq[:, 1:4:2, PAD:HI],
                    pred_log[1, 0:256, :].rearrange("(s p) c -> p s c", p=128))
        g.dma_start(E6q[:, 0:4:2, PAD:HI],
                    variance[2, 0:256, :].rearrange("(s p) c -> p s c", p=128))
        g.dma_start(E6q[:, 1:4:2, PAD:HI],
                    variance[3, 0:256, :].rearrange("(s p) c -> p s c", p=128))
        g.dma_start(
            E23S[0:96, 4 * FH:6 * FH].rearrange(
                "p (q c) -> p q c", q=2)[:, :, PAD:HI],
            variance[2:4, 256:352, :].rearrange("q r c -> r q c"))
        for i in range(3):
            _pad_memsets(nc, E0p[i], PAD, HI, FH, eng=g)
            _pad_memsets(nc, E1p[i], PAD, HI, FH, eng=g)

        segt = []
        for si, (r0, hs) in enumerate(RSEGS):
            m = mS[:, si * FH:(si + 1) * FH]
            p = p01S[:, 2 * si * FH:(2 * si + 1) * FH]
            D = DS[:, si * FH:(si + 1) * FH]
            E23 = E23S[:, 2 * si * FH:(2 * si + 2) * FH]
            E01 = E01S[:, 2 * si * FH:(2 * si + 2) * FH]
            Dh = t_("Dh", w=FHN, dt=BF16, bufs=2)
            u01 = t_("u01", 2 * FH, dt=BF16, bufs=2)
            Pp = t_("Pp", w=FHN, dt=F32, bufs=2)
            eN, eP = (t_("eN", w=FHN, dt=BF16, bufs=2),
                      t_("eP", w=FHN, dt=BF16, bufs=2))
            E0, E1 = E01[:, 0:FH], E01[:, FH:2 * FH]
            u0, u1 = u01[:, 0:FH], u01[:, FH:2 * FH]
            _pad_memsets(nc, u01[:, 0:FH], PAD, HI, FH)
            _pad_memsets(nc, u01[:, FH:2 * FH], PAD, HI, FH)

            _axis_prefix(nc, p, Pp, eN, eP, PAD, HI)
            # E = max(exp(-v), e^-5): in-place exp on Act, clamp on DVE
            e2 = E01[0:hs, 0:2 * FH].rearrange("p (s c) -> p s c", s=2)
            nc.scalar.activation(e2[:, :, PAD:HI], e2[:, :, PAD:HI],
                                 ActF.Exp, scale=-1.0)
            clamp_eng = v if si == 0 else g
            clamp_eng.tensor_scalar_max(
                E01[:, 0:2 * FH].rearrange("p (s c) -> p s c", s=2)[:, :, PAD:HI],
                E01[:, 0:2 * FH].rearrange("p (s c) -> p s c", s=2)[:, :, PAD:HI],
                EM5)
            if si > 0:
                _mw_chain(nc, m, Mw[si], Sm, mwc, PAD, HI)

            _axis_main(nc, m, p, E0, E1, D, Dh, u0, u1, Mw[si], c1, c2,
                       Pp, eN, eP, u0p, u1p, E0p[si], E1p[si], PAD, HI,
                       twdH[:, si * W:(si + 1) * W])

            # E23 for the V phase: exp in place (Act has slack), clamp
            # deferred to the V layout
            e23 = E23[0:hs, 0:2 * FH].rearrange("p (s c) -> p s c", s=2)
            nc.scalar.activation(e23[:, :, PAD:HI], e23[:, :, PAD:HI],
                                 ActF.Exp, scale=-1.0)
            segt.append((m, E0, E1, si))

        # V-direction 32-row window sums on PE (convolution over rows),
        # read out by Act, compared on GPSIMD into the row-major Mw plane
        for si, (r0, hs) in enumerate(RSEGS):
            m_si = mS[:, si * FH:(si + 1) * FH]
            for cc0, ccw in ((0, 512), (512, 512), (1024, 192)):
                pv = pp.tile([128, 512], F32, tag="pv", bufs=2, name="pv")
                if si == 0:
                    nc.tensor.matmul(pv[:, 0:ccw],
                                     W1[0:hs, :],
                                     m_si[0:hs, PAD + cc0:PAD + cc0 + ccw],
                                     start=True, stop=True)
                else:
                    m_pr = mS[:, (si - 1) * FH:si * FH]
                    nc.tensor.matmul(pv[:, 0:ccw],
                                     W1[0:hs, :],
                                     m_si[0:hs, PAD + cc0:PAD + cc0 + ccw],
                                     start=True, stop=False)
                    nc.tensor.matmul(pv[:, 0:ccw],
                                     W2[0:128, :],
                                     m_pr[0:128, PAD + cc0:PAD + cc0 + ccw],
                                     start=False, stop=True)
                nc.scalar.copy(
                    MwVH[:, si * FH + PAD + cc0:si * FH + PAD + cc0 + ccw],
                    pv[:, 0:ccw])
        mw3d = MwVH.rearrange("p (s c) -> p s c", s=3)
        g.tensor_scalar(mw3d[:, :, PAD:HI], mw3d[:, :, PAD:HI],
                        float(R) - 0.5, None, op0=Alu.is_ge)

        # deferred GPSIMD weight scans (need the pre-corrections), then the
        # aw totals on DVE once the scans land
        for m, E0, E1, si in segt:
            _axis_aw_scans(nc, m, E0p[si], E1p[si], E0, E1, PAD, HI)
        for m, E0, E1, si in segt:
            v.tensor_add(twH[:, si * W:(si + 1) * W],
                         E0[:, PAD:HI], E1[:, PAD:HI])



def _v_transposes(vin, mS, p01S, DS, E23S):
    """XBAR transpose call lists (early; late): one call per (plane,
    row-segment) covering the full 1280-col width -> 10 V col-groups.
    seg_base(si) gives the FH-slot offset of row-segment si in the source."""
    early, late = [], []

    def plane_calls(out, src, seg_base, dst):
        o10 = dst[:, PAD:PAD + 10 * VSEG].rearrange("p (s c) -> p s c", s=10)
        for rp, (r0, hs) in enumerate(RSEGS):
            b = seg_base(rp) + PAD
            out.append((o10[:, :, rp * 128:rp * 128 + hs],
                        src[0:hs, b:b + 1280]))

    plane_calls(early, mS, lambda s: s * FH, vin["vm"])
    plane_calls(early, p01S, lambda s: (2 * s + 1) * FH, vin["vq"])
    plane_calls(early, DS, lambda s: s * FH, vin["vD"])
    plane_calls(late, E23S, lambda s: 2 * s * FH, vin["vE0"])
    plane_calls(late, E23S, lambda s: (2 * s + 1) * FH, vin["vE1"])
    return early, late


def _vmw_transposes(MwVH, vMwB):
    """Mw rows include one extra fold for seg2 (rows 96..111): conv row 96
    holds the window ending at the image's last row, which the backward
    pre-correction reads at the first gap position; rows 97+ are real
    zeros (partial windows)."""
    calls = []
    o10 = vMwB[:, PAD:PAD + 10 * VSEG].rearrange("p (s c) -> p s c", s=10)
    for rp, (r0, hs) in enumerate(RSEGS):
        b = rp * FH + PAD
        he = 112 if rp == 2 else hs
        calls.append((o10[:, :, rp * 128:rp * 128 + he],
                      MwVH[0:he, b:b + 1280]))
    return calls


def _tpose_out_acc(nc, pp, ident, src, dst, cw, c0, eng=None):
    """Transposed src [128, FV] fp16 -> row-major: dst += src^T (in place).
    One PSUM-operand tensor_tensor add per merged group."""
    v = eng if eng is not None else nc.vector
    ncs = (cw + 127) // 128
    for rp, (r0, hs) in enumerate(RSEGS):
        cs = 0
        while cs < ncs:
            bw = min(128, cw - cs * 128)
            fb = PAD + cs * VSEG + rp * 128
            ng = 0
            while (cs + ng < ncs and ng < 4
                   and min(128, cw - (cs + ng) * 128) == 128):
                ng += 1
            cb = rp * W + c0 + cs * 128
            if ng >= 2:
                psu = pp.tile([128, 128 * ng], F16, tag="pt2", bufs=3,
                              name="psg")
                for gi in range(ng):
                    nc.tensor.transpose(
                        psu[0:hs, 128 * gi:128 * (gi + 1)],
                        src[:, fb + VSEG * gi:fb + VSEG * gi + hs],
                        ident[:, :])
                v.tensor_tensor(dst[0:hs, cb:cb + 128 * ng],
                                psu[0:hs, 0:128 * ng],
                                dst[0:hs, cb:cb + 128 * ng], op=Alu.add)
                cs += ng
            else:
                psu = pp.tile([128, 128], F16, tag="pt1", bufs=2)
                nc.tensor.transpose(psu[0:hs, 0:bw], src[0:bw, fb:fb + hs],
                                    ident[0:bw, 0:bw])
                v.tensor_tensor(dst[0:hs, cb:cb + bw], psu[0:hs, 0:bw],
                                dst[0:hs, cb:cb + bw], op=Alu.add)
                cs += 1


def _v_phase(nc, tc, pp, ident, depth_orig, vin, twH, twdH, vMwB, bl):
    v = nc.vector
    g = nc.gpsimd
    lo = PAD
    with tc.tile_pool(name="vp", bufs=1) as vp:
        def t_(tag, dt=F16, bufs=1, w=FV):
            return vp.tile([128, w], dt, tag=tag, name=tag, bufs=bufs)

        u0, u1 = t_("vu0", BF16), t_("vu1", BF16)
        c1, c2 = t_("vc1", BF16), t_("vc2", BF16)
        Pp = t_("vPp", F32, w=VHI + 2)
        eN, eP = t_("veN", BF16, w=VHI + 2), t_("veP", BF16, w=VHI + 2)
        u0p, u1p = t_("vu0p", BF16), t_("vu1p", BF16)
        Dh = t_("vDh", BF16, w=VHI + 2)
        E0p, E1p = t_("vE0p"), t_("vE1p")
        awd, aw = t_("vawd"), t_("vaw")
        selB = t_("selB", w=3 * 640)
        rcpB = t_("rcpB", BF16, w=3 * 640)
        outO = t_("outO", F32, w=3 * 640)
        DoC = [t_(f"DoC{i}", w=3 * 640) for i in range(2)]
        for t in (u0, u1, u0p, u1p):
            _pad_memsets(nc, t, lo, VHI, FV)
        for t in (E0p, E1p):
            _pad_memsets(nc, t, lo, VHI, FV, eng=g)
        def vw(nm, ci):
            off = 5 * VSEG * ci
            return vin[nm][:, off:off + FV]

        Mw = [vMwB[:, 0:FV], vMwB[:, 5 * VSEG:5 * VSEG + FV]]

        # E clamp deferred from the H phase, applied on the whole V planes
        # in GPSIMD's idle window (gap zeros clamp to e^-5; harmless, the
        # mask gap kills those paths)
        ce = PAD + 10 * VSEG
        g.tensor_scalar_max(vin["vE0"][:, lo:ce], vin["vE0"][:, lo:ce], EM5)
        g.tensor_scalar_max(vin["vE1"][:, lo:ce], vin["vE1"][:, lo:ce], EM5)
        # depth_orig loads for the blend (Pool has slack by now)
        for ci, (c0, cw) in enumerate(VCHUNKS):
            g.dma_start(
                DoC[ci][:, 0:2 * 640].rearrange(
                    "p (s c) -> p s c", s=2)[:, :, 0:cw],
                depth_orig[0, 0:256, c0:c0 + cw].rearrange(
                    "(s p) c -> p s c", p=128))
            g.dma_start(DoC[ci][0:96, 2 * 640:2 * 640 + cw],
                        depth_orig[0, 256:352, c0:c0 + cw])
            g.memset(DoC[ci][96:128, 2 * 640:3 * 640], 0.0)

        pending = None
        for ci, (c0, cw) in enumerate(VCHUNKS):
            m, q = vw("vm", ci), vw("vq", ci)
            D = vw("vD", ci)
            E0, E1 = vw("vE0", ci), vw("vE1", ci)
            _axis_prefix(nc, q, Pp, eN, eP, lo, VHI)
            _axis_main(nc, m, q, E0, E1, D, Dh, u0, u1, Mw[ci], c1, c2,
                       Pp, eN, eP, u0p, u1p, E0p, E1p, lo, VHI,
                       awd[:, lo:VHI], esub_eng=nc.gpsimd)
            _axis_aw_scans(nc, m, E0p, E1p, E0, E1, lo, VHI)
            g.tensor_tensor(aw[:, lo:VHI], E0[:, lo:VHI], E1[:, lo:VHI],
                            op=Alu.add)
            # transpose back through PE with the H+V accumulation fused into
            # the PSUM-read add (in place on twdH/twH)
            _tpose_out_acc(nc, pp, ident, awd, twdH, cw, c0)
            _tpose_out_acc(nc, pp, ident, aw, twH, cw, c0)
            if pending is not None:
                _blend_chunk(nc, bl, DoC[pending[2]], selB, rcpB, outO, *pending[:2])
            pending = (c0, cw, ci)
        # split the final chunk's blend so earlier pieces' stores overlap
        # later pieces' compute (shorter tail)
        c0, cw, ci = pending
        h1 = (cw // 2 + 15) // 16 * 16
        h2 = (3 * cw // 4 + 15) // 16 * 16
        _blend_chunk(nc, bl, DoC[ci], selB, rcpB, outO, c0, h1)
        _blend_chunk(nc, bl, DoC[ci], selB, rcpB, outO, c0 + h1, h2 - h1, off=h1)
        _blend_chunk(nc, bl, DoC[ci], selB, rcpB, outO, c0 + h2, cw - h2, off=h2)


def _blend_chunk(nc, bl, Do, selB, rcpB, outO, c0, cw, off=0):
    """Final blend for image columns [c0, c0+cw) on row-major planes.
    Do/selB/outO are chunk-relative [128, 3, 640] views at offset off."""
    v = nc.vector
    lo = PAD

    def cs(t):
        return t[:, 0:3 * W].rearrange("p (s c) -> p s c", s=3)[:, :, c0:c0 + cw]

    def cr(t):
        return t[:, 0:3 * 640].rearrange("p (s c) -> p s c", s=3)[:, :, off:off + cw]

    mS, twH, twdH = bl["mS"], bl["twH"], bl["twdH"]
    lam_t = bl["lam_t"]
    msk = mS.rearrange("p (s c) -> p s c", s=3)[:, :, lo + c0:lo + c0 + cw]
    nc.gpsimd.tensor_scalar(cr(selB), cs(twH), 0.0, None, op0=Alu.is_gt)
    v.tensor_mul(cr(selB), cr(selB), msk)
    nc.gpsimd.tensor_scalar_max(cs(twH), cs(twH), 1e-6)
    nc.scalar.activation(cr(selB), cr(selB), ActF.Copy, scale=lam_t[:, 0:1])
    # 1/tw via exp(-ln(tw)) on Act (set 6 holds both exp and ln: no swaps)
    nc.scalar.activation(cr(rcpB), cs(twH), ActF.Ln)
    nc.scalar.activation(cr(rcpB), cr(rcpB), ActF.Exp, scale=-1.0)
    v.tensor_mul(cs(twdH), cs(twdH), cr(rcpB))       # lat = twd / tw
    v.tensor_sub(cs(twdH), cs(twdH), cr(Do))         # lat - Do
    v.tensor_mul(cs(twdH), cs(twdH), cr(selB))       # * sel * lam
    v.tensor_tensor(cr(outO), cr(Do), cs(twdH), op=Alu.add)
    for si, (r0, hs) in enumerate(RSEGS):
        rs = slice(r0, r0 + hs)
        nc.sync.dma_start(
            bl["depthout"][0, rs, c0:c0 + cw],
            outO[0:hs, si * 640 + off:si * 640 + off + cw])


_NC = None


def _get_nc():
    global _NC
    if _NC is None:
        _NC = build_program()
    return _NC


def kernel(pred_log, mask, variance, depthin, lam, times):
    pred_log = np.ascontiguousarray(np.asarray(pred_log, dtype=np.float32))
    mask = np.ascontiguousarray(np.asarray(mask, dtype=np.int32))
    variance = np.ascontiguousarray(np.asarray(variance, dtype=np.float32))
    depthin = np.ascontiguousarray(np.asarray(depthin, dtype=np.float32))
    lam = np.ascontiguousarray(np.asarray(lam, dtype=np.float32)).reshape(1)
    t = int(np.asarray(times))

    if t <= 0:
        return depthin.copy()
    nc = _get_nc()
    depth_cur = depthin
    for _ in range(t):
        in_maps = [{
            "pred_log": pred_log[b],
            "mask": mask[b],
            "variance": variance[b],
            "depth_cur": depth_cur[b],
            "depth_orig": depthin[b],
            "lam": lam,
        } for b in range(B)]
        res = run_bass_kernel_spmd(nc, in_maps, list(range(B)))
        depth_cur = np.stack([res.results[i]["depthout"] for i in range(B)])
    return depth_cur.astype(np.float32)


# revision 78
# speedup vs baseline: 1.0040x; 1.0040x over previous
"""CRF integration (nn_CRFIntegrationModule) Trainium2 kernel.

One image per NeuronCore (B=8 -> 8 cores).  Each direction's 32-step windowed
scan is one hardware tensor_tensor_scan on a pre-corrected input (windowed
linear recurrence):

    A32[n] = (A32[n-1] + u'[n-1]) * m[n-1]
    u'[k]  = u[k] - M[k] * u[k -/+ 32]        (M = windowed mask product)

The per-path exponent exp(sum plog) is factored out through the global plog
prefix P (one scan):  awd = e^{P} * scan(u * e^{-P}, m), so every scan
transition is the exact mask and no exp factors compound along paths.
M comes from a mask cumsum scan + windowed difference + ==32 compare, all on
GPSIMD (one engine-local chain).

Data flow: the H phase loads every input plane once (SWDGE cast loads to
fp16, multi-plane fused calls, all enqueued up front in readiness order —
engine queues are in-order FIFOs, so issue order is chosen to match expected
operand arrival) into persistent row-major SBUF planes and runs the
horizontal scans.  The V phase reads its inputs from those SBUF planes
through the XBAR DMA transpose (issued on the otherwise idle SP sequencer:
no compute engine spends time staging or transposing), scans vertically, and
PE-transposes the results back, accumulating into the H totals in place.
Work is split across DVE / GPSIMD / Act so every stream stays busy: the
mask-window machinery and the aw (weight) scans run on GPSIMD, deferred
until their inputs are ready so they never head-block the queue; the final
blend divides on GPSIMD (no Ln/Exp activation-table swaps anywhere).
"""
import os
import sys

for _p in ("/opt/trn_rl_repo", "/root/.axon_site/_ro/trn_rl_repo"):
    if os.path.isdir(_p) and _p not in sys.path:
        sys.path.insert(0, _p)
        break

import numpy as np
import concourse.bacc as bacc
import concourse.mybir as mybir
import concourse.tile as tile
from concourse import masks
from concourse.bass_utils import run_bass_kernel_spmd

Alu = mybir.AluOpType
ActF = mybir.ActivationFunctionType
F32 = mybir.dt.float32
I32 = mybir.dt.int32
BF16 = mybir.dt.bfloat16
F16 = mybir.dt.float16

B, H, W = 8, 352, 1216
R = 32          # MAXRANGE
CLIP = 5.0      # CLIPVARIANCE
EM5 = float(np.exp(-CLIP))
PAD = 32

# H-phase geometry: row segments (partitions = rows), padded row-major planes
RSEGS = [(0, 128), (128, 128), (256, 96)]          # (row0, height)
FH = PAD + 1280                                    # transposes read 1280 wide
HI = PAD + W
FHN = HI + 2                                       # narrow scratch width
# V-phase geometry: transposed layout, 2 chunks x 5 col-segments of <=128 cols
VSEG = H + PAD                                     # 384 per col-seg span
NCS = 5
FV = PAD + NCS * VSEG + 8                          # 1960
VHI = PAD + (NCS - 1) * VSEG + H                   # 1920
FVB = PAD + 2 * NCS * VSEG + 8                     # 3880: both chunks
VCHUNKS = [(0, 640), (640, 576)]


def _pad_memsets(nc, t, lo, hi, F, vgaps=False, eng=None):
    """Zero the pad strips of a [128, F] tile (head, tail, V inter-seg gaps)."""
    g_ = eng if eng is not None else nc.vector
    g_.memset(t[:, 0:lo], 0.0)
    g_.memset(t[:, hi:F], 0.0)
    if vgaps:
        g = t[:, PAD:PAD + 5 * VSEG].rearrange("p (s c) -> p s c", s=5)
        g_.memset(g[:, :, H:VSEG], 0.0)


def _mw_chain(nc, m, Mw, Sm, cs, lo, hi, sub_eng=None):
    """Windowed mask product M[n] = prod_{j=1..32} m[n-j]: cumsum scan (DVE:
    hardware scans only exist there) + windowed difference + ==32 compare."""
    g = nc.gpsimd
    sub = sub_eng if sub_eng is not None else g
    nc.vector.tensor_tensor_scan(
        Sm[:, 1:hi + 33], m[:, 0:hi + 32], m[:, 0:hi + 32],
        0.0, op0=Alu.add, op1=Alu.bypass)
    sub.tensor_tensor(cs[:, lo:hi + 33], Sm[:, lo:hi + 33],
                      Sm[:, lo - R:hi + 33 - R], op=Alu.subtract)
    g.tensor_scalar(Mw[:, lo:hi + 33], cs[:, lo:hi + 33], float(R) - 0.5,
                    None, op0=Alu.is_ge)


def _axis_prefix(nc, p, Pp, eN, eP, lo, hi):
    """Plog prefix scan + its exponentials: first DVE/Act work of a segment
    (issued before the E-plane exp/clamp so it never waits on them)."""
    nc.vector.tensor_tensor_scan(
        Pp[:, lo:hi], p[:, lo - 1:hi - 1], p[:, lo - 1:hi - 1], 0.0,
        op0=Alu.add, op1=Alu.bypass)
    nc.scalar.activation(eN[:, lo:hi], Pp[:, lo:hi], ActF.Exp, scale=-1.0)
    nc.scalar.activation(eP[:, lo:hi], Pp[:, lo:hi], ActF.Exp)


def _axis_main(nc, m, p, E0, E1, D, Dh, u0, u1, Mw, c1, c2,
               Pp, eN, eP, u0p, u1p, E0p, E1p, lo, hi, awd_out,
               esub_eng=None):
    """DVE stream of one axis pair: u-hat, windowed pre-corrections, the two
    awd scans, and the awd output.  The aw (weight) scans BL/BR run on GPSIMD
    and are emitted separately via _axis_aw_scans; prefix via _axis_prefix;
    scan outputs alias u0/u1/E0/E1."""
    v = nc.vector
    sc = v.tensor_tensor_scan
    # u-hat = E * D * e^{-P}
    v.tensor_mul(Dh[:, lo:hi], D[:, lo:hi], eN[:, lo:hi])
    v.tensor_mul(u0[:, lo:hi], E0[:, lo:hi], Dh[:, lo:hi])
    v.tensor_mul(u1[:, lo:hi], E1[:, lo:hi], Dh[:, lo:hi])
    # windowed pre-corrections  u'[m] = u[m] - M * u[m -/+ 32]  (mask-only)
    v.tensor_mul(c1[:, lo:hi], Mw[:, lo:hi], u0[:, lo - R:hi - R])
    v.tensor_sub(u0p[:, lo:hi], u0[:, lo:hi], c1[:, lo:hi])
    v.tensor_mul(c2[:, lo:hi], Mw[:, lo + R + 1:hi + R + 1],
                 u1[:, lo + R:hi + R])
    v.tensor_sub(u1p[:, lo:hi], u1[:, lo:hi], c2[:, lo:hi])
    es = esub_eng if esub_eng is not None else v
    v.tensor_mul(c1[:, lo:hi], Mw[:, lo:hi], E0[:, lo - R:hi - R])
    es.tensor_tensor(E0p[:, lo:hi], E0[:, lo:hi], c1[:, lo:hi],
                     op=Alu.subtract)
    v.tensor_mul(c2[:, lo:hi], Mw[:, lo + R + 1:hi + R + 1],
                 E1[:, lo + R:hi + R])
    es.tensor_tensor(E1p[:, lo:hi], E1[:, lo:hi], c2[:, lo:hi],
                     op=Alu.subtract)
    AL, AR = u0, u1
    sc(AL[:, lo:hi], u0p[:, lo - 1:hi - 1], m[:, lo - 1:hi - 1], 0.0,
       op0=Alu.add, op1=Alu.mult)
    sc(AR[:, lo:hi][:, ::-1], u1p[:, lo + 1:hi + 1][:, ::-1],
       m[:, lo + 1:hi + 1][:, ::-1], 0.0, op0=Alu.add, op1=Alu.mult)
    v.tensor_add(c1[:, lo:hi], AL[:, lo:hi], AR[:, lo:hi])
    v.tensor_mul(awd_out, c1[:, lo:hi], eP[:, lo:hi])


def _axis_aw_scans(nc, m, E0p, E1p, BL, BR, lo, hi, eng=None):
    """Deferred weight scans (aw numerator); DVE (scans are DVE-only)."""
    g = nc.vector
    g.tensor_tensor_scan(BL[:, lo:hi], E0p[:, lo - 1:hi - 1],
                         m[:, lo - 1:hi - 1], 0.0, op0=Alu.add, op1=Alu.mult)
    g.tensor_tensor_scan(BR[:, lo:hi][:, ::-1], E1p[:, lo + 1:hi + 1][:, ::-1],
                         m[:, lo + 1:hi + 1][:, ::-1], 0.0,
                         op0=Alu.add, op1=Alu.mult)


def build_program():
    nc = bacc.Bacc("TRN2", target_bir_lowering=False, debug=False, dynamic_dma_scratch_size=16384)

    pred_log = nc.dram_tensor("pred_log", [2, H, W], F32, kind="ExternalInput").ap()
    mask = nc.dram_tensor("mask", [1, H, W], I32, kind="ExternalInput").ap()
    variance = nc.dram_tensor("variance", [4, H, W], F32, kind="ExternalInput").ap()
    depth_cur = nc.dram_tensor("depth_cur", [1, H, W], F32, kind="ExternalInput").ap()
    depth_orig = nc.dram_tensor("depth_orig", [1, H, W], F32, kind="ExternalInput").ap()
    lam = nc.dram_tensor("lam", [1], F32, kind="ExternalInput").ap()
    depthout = nc.dram_tensor("depthout", [1, H, W], F32, kind="ExternalOutput").ap()

    g = nc.gpsimd
    lo = PAD
    with tile.TileContext(nc, pool_alloc_mode="queue") as tc:
        with tc.tile_pool(name="const", bufs=1) as cp, \
             tc.tile_pool(name="persist", bufs=1) as ps, \
             tc.tile_pool(name="psum", bufs=8, space="PSUM") as pp:
            ident = cp.tile([128, 128], F16, tag="ident")
            masks.make_identity(nc, ident[:])
            # natural_log_exp_and_others: covers every activation we use, so
            # the act-table pass never needs to insert mid-kernel swaps
            nc.scalar.add_instruction(mybir.InstLoadActFuncSet(
                name=nc.get_next_instruction_name(), act_func_set_id=6,
                ins=[], outs=[]))
            # band matrices: the V-direction 32-row window sum is a PE
            # convolution over rows in the row-major layout
            # W1[k, i] = 1 iff i-32 <= k < i (own segment rows)
            W1 = cp.tile([128, 128], F16, tag="W1")
            nc.gpsimd.memset(W1[:, :], 1.0)
            nc.gpsimd.affine_select(W1[:, :], W1[:, :], base=32,
                                    channel_multiplier=1,
                                    pattern=[[-1, 128]],
                                    compare_op=Alu.is_ge, fill=0.0)
            nc.gpsimd.affine_select(W1[:, :], W1[:, :], base=-1,
                                    channel_multiplier=-1,
                                    pattern=[[1, 128]],
                                    compare_op=Alu.is_ge, fill=0.0)
            # W2[k, i] = 1 iff k >= 96 + i (previous segment's tail rows)
            W2 = cp.tile([128, 128], F16, tag="W2")
            nc.gpsimd.memset(W2[:, :], 1.0)
            nc.gpsimd.affine_select(W2[:, :], W2[:, :], base=-96,
                                    channel_multiplier=1,
                                    pattern=[[-1, 128]],
                                    compare_op=Alu.is_ge, fill=0.0)
            lam_t = cp.tile([128, 1], F32, tag="lam")

            # persistent row-major fp16 planes, 3 row-seg slots of width FH
            mS = ps.tile([128, 3 * FH], F16, tag="mS")
            p01S = ps.tile([128, 6 * FH], F16, tag="p01S")    # [seg][p0,p1]
            DS = ps.tile([128, 3 * FH], F16, tag="DS")
            E23S = ps.tile([128, 6 * FH], F16, tag="E23S")    # [seg][E2,E3]
            twH = ps.tile([128, 3 * W], F16, tag="twH")
            twdH = ps.tile([128, 3 * W], F16, tag="twdH")
            # V input planes (written by SP DMA transposes; pads and
            # inter-seg gaps zeroed in the prologue while DVE is idle).
            # One full-width tile per plane: col-groups 0-4 are V chunk 0,
            # 5-9 chunk 1; each chunk's compute uses a [128, FV] window.
            vin = {}
            for nm in ("vm", "vq", "vD", "vE0", "vE1"):
                vin[nm] = ps.tile([128, FVB], F16, tag=f"{nm}B",
                                  name=f"{nm}B")
            MwVH = ps.tile([128, 3 * FH], F16, tag="MwVH")
            vMwB = ps.tile([128, FVB], F16, tag="vMwB")

            # ---- prologue ------------------------------------------------
            # pad zeroing first (everything idle at t=0; loads write only
            # the interiors, but region tracking may be whole-tile, so pads
            # go first to keep the load DMAs unblocked)
            m3 = mS.rearrange("p (s c) -> p s c", s=3)
            p6 = p01S.rearrange("p (s c) -> p s c", s=6)
            D3 = DS.rearrange("p (s c) -> p s c", s=3)
            E6 = E23S.rearrange("p (s c) -> p s c", s=6)
            for t3 in (p6, E6):
                g.memset(t3[:, :, 0:PAD], 0.0)
                g.memset(t3[:, :, HI:FH], 0.0)
            for t3 in (m3, D3):
                nc.vector.memset(t3[:, :, 0:PAD], 0.0)
                nc.vector.memset(t3[:, :, HI:FH], 0.0)
            # seg2 slots hold 96 rows; full-partition ops read rows 96:127
            g.memset(p6[96:128, 4:6, PAD:HI], 0.0)
            nc.vector.memset(m3[96:128, 2:3, PAD:HI], 0.0)
            nc.vector.memset(D3[96:128, 2:3, PAD:HI], 0.0)
            for t in list(vin.values()) + [vMwB]:
                nc.vector.memset(t[:, 0:lo], 0.0)
                nc.vector.memset(t[:, PAD + 10 * VSEG:FVB], 0.0)
                gv = t[:, PAD:PAD + 10 * VSEG].rearrange(
                    "p (s c) -> p s c", s=10)
                nc.vector.memset(gv[:, :, H:VSEG], 0.0)
            mw3 = MwVH.rearrange("p (s c) -> p s c", s=3)
            nc.vector.memset(mw3[:, :, 0:PAD], 0.0)
            nc.vector.memset(mw3[:, :, HI:FH], 0.0)


            _h_phase(nc, tc, mask, pred_log, variance, depth_cur,
                     depth_orig, mS, p01S, DS, E23S, twH, twdH, m3, D3,
                     pp, W1, W2, MwVH)

            # V-input transposes on the idle SP sequencer.  The manual wait
            # overrides keep them out of the early DMA window (the framework
            # serializes the in-flight DMA stream, so transposes scheduled
            # between the critical H loads would push those loads out).
            early, late = _v_transposes(vin, mS, p01S, DS, E23S)
            with tc.tile_wait_until(0.036):
                for args in early:
                    nc.sync.dma_start_transpose(*args)
            with tc.tile_wait_until(0.040):
                for args in late:
                    nc.sync.dma_start_transpose(*args)

            with tc.tile_wait_until(0.040):
                for args in _vmw_transposes(MwVH, vMwB):
                    nc.sync.dma_start_transpose(*args)

            with tc.tile_wait_until(0.040):
                nc.sync.dma_start(lam_t[:, 0:1], lam.partition_broadcast(128))
            bl = dict(mS=mS, twH=twH, twdH=twdH, lam_t=lam_t,
                      depthout=depthout)
            _v_phase(nc, tc, pp, ident, depth_orig, vin, twH, twdH, vMwB, bl)
    nc.finalize()
    return nc


def _h_phase(nc, tc, mask, pred_log, variance, depth, depth_orig,
             mS, p01S, DS, E23S, twH, twdH, m3, D3, pp, W1, W2, MwVH):
    v = nc.vector
    g = nc.gpsimd
    with tc.tile_pool(name="hp", bufs=1) as hp:
        def t_(tag, w=FH, dt=F16, bufs=1):
            return hp.tile([128, w], dt, tag=tag, name=tag, bufs=bufs)

        c1, c2 = t_("c1", dt=BF16), t_("c2", dt=BF16)
        mwc = t_("mwc", dt=BF16)
        u0p, u1p = t_("u0p", dt=BF16), t_("u1p", dt=BF16)
        E0p = [t_(f"E0p{i}") for i in range(3)]
        E1p = [t_(f"E1p{i}") for i in range(3)]
        Mw = [t_(f"Mw{i}") for i in range(3)]
        Sm = t_("Sm")
        E01S = t_("E01S", 6 * FH)                      # [seg][E0,E1]
        for t in (u0p, u1p):
            _pad_memsets(nc, t, PAD, HI, FH)
        e6 = E01S.rearrange("p (s c) -> p s c", s=6)
        nc.vector.memset(e6[:, :, 0:PAD], 0.0)
        nc.vector.memset(e6[:, :, HI:FH], 0.0)
        nc.vector.memset(e6[96:128, 4:6, PAD:HI], 0.0)
        g.memset(Sm[:, 0:1], 0.0)

        # ---- load batch 1: seg0+1 essentials (free DMA-window slots) ----
        pq = p01S.rearrange("p (s c) -> p s c", s=6)
        E6q = E23S.rearrange("p (s c) -> p s c", s=6)
        g.dma_start(pq[:, 0:4:2, PAD:HI],
                    pred_log[0, 0:256, :].rearrange("(s p) c -> p s c", p=128))
        g.dma_start(m3[:, 0:2, PAD:HI],
                    mask[0, 0:256, :].rearrange("(s p) c -> p s c", p=128))
        g.dma_start(e6[:, 0:4:2, PAD:HI],
                    variance[0, 0:256, :].rearrange("(s p) c -> p s c", p=128))
        g.dma_start(e6[:, 1:4:2, PAD:HI],
                    variance[1, 0:256, :].rearrange("(s p) c -> p s c", p=128))
        g.dma_start(D3[:, 0:2, PAD:HI],
                    depth[0, 0:256, :].rearrange("(s p) c -> p s c", p=128))

        # seg0's mask-window chain slots between the two enqueue batches, so
        # the later enqueues' DMA-window waits are already satisfied when the
        # Pool engine reaches them (no head-of-line stall either way)
        _mw_chain(nc, mS[:, 0:FH], Mw[0], Sm, mwc, PAD, HI)

        # ---- load batch 2: seg2 + the V-phase E planes -------------------
        g.dma_start(mS[0:96, 2 * FH + PAD:2 * FH + HI], mask[0, 256:352, :])
        g.dma_start(
            p01S[0:96, 4 * FH:6 * FH].rearrange(
                "p (q c) -> p q c", q=2)[:, :, PAD:HI],
            pred_log[0:2, 256:352, :].rearrange("q r c -> r q c"))
        g.dma_start(DS[0:96, 2 * FH + PAD:2 * FH + HI],
                    depth[0, 256:352, :])
        g.dma_start(
            E01S[0:96, 4 * FH:6 * FH].rearrange(
                "p (q c) -> p q c", q=2)[:, :, PAD:HI],
            variance[0:2, 256:352, :].rearrange("q r c -> r q c"))
        g.dma_start(pq[:, 1:4:2, PAD:HI],
                    pred_log[1, 0:256, :].rearrange("(s p) c -> p s c", p=128))
        g.dma_start(E6q[:, 0:4:2, PAD:HI],
                    variance[2, 0:256, :].rearrange("(s p) c -> p s c", p=128))
        g.dma_start(E6q[:, 1:4:2, PAD:HI],
                    variance[3, 0:256, :].rearrange("(s p) c -> p s c", p=128))
        g.dma_start(
            E23S[0:96, 4 * FH:6 * FH].rearrange(
                "p (q c) -> p q c", q=2)[:, :, PAD:HI],
            variance[2:4, 256:352, :].rearrange("q r c -> r q c"))
        for i in range(3):
            _pad_memsets(nc, E0p[i], PAD, HI, FH, eng=g)
            _pad_memsets(nc, E1p[i], PAD, HI, FH, eng=g)

        segt = []
        for si, (r0, hs) in enumerate(RSEGS):
            m = mS[:, si * FH:(si + 1) * FH]
            p = p01S[:, 2 * si * FH:(2 * si + 1) * FH]
            D = DS[:, si * FH:(si + 1) * FH]
            E23 = E23S[:, 2 * si * FH:(2 * si + 2) * FH]
            E01 = E01S[:, 2 * si * FH:(2 * si + 2) * FH]
            Dh = t_("Dh", w=FHN, dt=BF16, bufs=2)
            u01 = t_("u01", 2 * FH, dt=BF16, bufs=2)
            Pp = t_("Pp", w=FHN, dt=F32, bufs=2)
            eN, eP = (t_("eN", w=FHN, dt=BF16, bufs=2),
                      t_("eP", w=FHN, dt=BF16, bufs=2))
            E0, E1 = E01[:, 0:FH], E01[:, FH:2 * FH]
            u0, u1 = u01[:, 0:FH], u01[:, FH:2 * FH]
            _pad_memsets(nc, u01[:, 0:FH], PAD, HI, FH)
            _pad_memsets(nc, u01[:, FH:2 * FH], PAD, HI, FH)

            _axis_prefix(nc, p, Pp, eN, eP, PAD, HI)
            # E = max(exp(-v), e^-5): in-place exp on Act, clamp on DVE
            e2 = E01[0:hs, 0:2 * FH].rearrange("p (s c) -> p s c", s=2)
            nc.scalar.activation(e2[:, :, PAD:HI], e2[:, :, PAD:HI],
                                 ActF.Exp, scale=-1.0)
            clamp_eng = v if si == 0 else g
            clamp_eng.tensor_scalar_max(
                E01[:, 0:2 * FH].rearrange("p (s c) -> p s c", s=2)[:, :, PAD:HI],
                E01[:, 0:2 * FH].rearrange("p (s c) -> p s c", s=2)[:, :, PAD:HI],
                EM5)
            if si > 0:
                _mw_chain(nc, m, Mw[si], Sm, mwc, PAD, HI)

            _axis_main(nc, m, p, E0, E1, D, Dh, u0, u1, Mw[si], c1, c2,
                       Pp, eN, eP, u0p, u1p, E0p[si], E1p[si], PAD, HI,
                       twdH[:, si * W:(si + 1) * W])

            # E23 for the V phase: exp in place (Act has slack), clamp
            # deferred to the V layout
            e23 = E23[0:hs, 0:2 * FH].rearrange("p (s c) -> p s c", s=2)
            nc.scalar.activation(e23[:, :, PAD:HI], e23[:, :, PAD:HI],
                                 ActF.Exp, scale=-1.0)
            segt.append((m, E0, E1, si))

        # V-direction 32-row window sums on PE (convolution over rows),
        # read out by Act, compared on GPSIMD into the row-major Mw plane
        for si, (r0, hs) in enumerate(RSEGS):
            m_si = mS[:, si * FH:(si + 1) * FH]
            for cc0, ccw in ((0, 512), (512, 512), (1024, 192)):
                pv = pp.tile([128, 512], F32, tag="pv", bufs=2, name="pv")
                if si == 0:
                    nc.tensor.matmul(pv[:, 0:ccw],
                                     W1[0:hs, :],
                                     m_si[0:hs, PAD + cc0:PAD + cc0 + ccw],
                                     start=True, stop=True)
                else:
                    m_pr = mS[:, (si - 1) * FH:si * FH]
                    nc.tensor.matmul(pv[:, 0:ccw],
                                     W1[0:hs, :],
                                     m_si[0:hs, PAD + cc0:PAD + cc0 + ccw],
                                     start=True, stop=False)
                    nc.tensor.matmul(pv[:, 0:ccw],
                                     W2[0:128, :],
                                     m_pr[0:128, PAD + cc0:PAD + cc0 + ccw],
                                     start=False, stop=True)
                nc.scalar.copy(
                    MwVH[:, si * FH + PAD + cc0:si * FH + PAD + cc0 + ccw],
                    pv[:, 0:ccw])
        mw3d = MwVH.rearrange("p (s c) -> p s c", s=3)
        g.tensor_scalar(mw3d[:, :, PAD:HI], mw3d[:, :, PAD:HI],
                        float(R) - 0.5, None, op0=Alu.is_ge)

        # deferred GPSIMD weight scans (need the pre-corrections), then the
        # aw totals on DVE once the scans land
        for m, E0, E1, si in segt:
            _axis_aw_scans(nc, m, E0p[si], E1p[si], E0, E1, PAD, HI)
        for m, E0, E1, si in segt:
            v.tensor_add(twH[:, si * W:(si + 1) * W],
                         E0[:, PAD:HI], E1[:, PAD:HI])



def _v_transposes(vin, mS, p01S, DS, E23S):
    """XBAR transpose call lists (early; late): one call per (plane,
    row-segment) covering the full 1280-col width -> 10 V col-groups.
    seg_base(si) gives the FH-slot offset of row-segment si in the source."""
    early, late = [], []

    def plane_calls(out, src, seg_base, dst):
        o10 = dst[:, PAD:PAD + 10 * VSEG].rearrange("p (s c) -> p s c", s=10)
        for rp, (r0, hs) in enumerate(RSEGS):
            b = seg_base(rp) + PAD
            out.append((o10[:, :, rp * 128:rp * 128 + hs],
                        src[0:hs, b:b + 1280]))

    plane_calls(early, mS, lambda s: s * FH, vin["vm"])
    plane_calls(early, p01S, lambda s: (2 * s + 1) * FH, vin["vq"])
    plane_calls(early, DS, lambda s: s * FH, vin["vD"])
    plane_calls(late, E23S, lambda s: 2 * s * FH, vin["vE0"])
    plane_calls(late, E23S, lambda s: (2 * s + 1) * FH, vin["vE1"])
    return early, late


def _vmw_transposes(MwVH, vMwB):
    """Mw rows include one extra fold for seg2 (rows 96..111): conv row 96
    holds the window ending at the image's last row, which the backward
    pre-correction reads at the first gap position; rows 97+ are real
    zeros (partial windows)."""
    calls = []
    o10 = vMwB[:, PAD:PAD + 10 * VSEG].rearrange("p (s c) -> p s c", s=10)
    for rp, (r0, hs) in enumerate(RSEGS):
        b = rp * FH + PAD
        he = 112 if rp == 2 else hs
        calls.append((o10[:, :, rp * 128:rp * 128 + he],
                      MwVH[0:he, b:b + 1280]))
    return calls


def _tpose_out_acc(nc, pp, ident, src, dst, cw, c0, eng=None):
    """Transposed src [128, FV] fp16 -> row-major: dst += src^T (in place).
    One PSUM-operand tensor_tensor add per merged group."""
    v = eng if eng is not None else nc.vector
    ncs = (cw + 127) // 128
    for rp, (r0, hs) in enumerate(RSEGS):
        cs = 0
        while cs < ncs:
            bw = min(128, cw - cs * 128)
            fb = PAD + cs * VSEG + rp * 128
            ng = 0
            while (cs + ng < ncs and ng < 4
                   and min(128, cw - (cs + ng) * 128) == 128):
                ng += 1
            cb = rp * W + c0 + cs * 128
            if ng >= 2:
                psu = pp.tile([128, 128 * ng], F16, tag="pt2", bufs=3,
                              name="psg")
                for gi in range(ng):
                    nc.tensor.transpose(
                        psu[0:hs, 128 * gi:128 * (gi + 1)],
                        src[:, fb + VSEG * gi:fb + VSEG * gi + hs],
                        ident[:, :])
                v.tensor_tensor(dst[0:hs, cb:cb + 128 * ng],
                                psu[0:hs, 0:128 * ng],
                                dst[0:hs, cb:cb + 128 * ng], op=Alu.add)
                cs += ng
            else:
                psu = pp.tile([128, 128], F16, tag="pt1", bufs=2)
                nc.tensor.transpose(psu[0:hs, 0:bw], src[0:bw, fb:fb + hs],
                                    ident[0:bw, 0:bw])
                v.tensor_tensor(dst[0:hs, cb:cb + bw], psu[0:hs, 0:bw],
                                dst[0:hs, cb:cb + bw], op=Alu.add)
                cs += 1


def _v_phase(nc, tc, pp, ident, depth_orig, vin, twH, twdH, vMwB, bl):
    v = nc.vector
    g = nc.gpsimd
    lo = PAD
    with tc.tile_pool(name="vp", bufs=1) as vp:
        def t_(tag, dt=F16, bufs=1, w=FV):
            return vp.tile([128, w], dt, tag=tag, name=tag, bufs=bufs)

        u0, u1 = t_("vu0", BF16), t_("vu1", BF16)
        c1, c2 = t_("vc1", BF16), t_("vc2", BF16)
        Pp = t_("vPp", F32, w=VHI + 2)
        eN, eP = t_("veN", BF16, w=VHI + 2), t_("veP", BF16, w=VHI + 2)
        u0p, u1p = t_("vu0p", BF16), t_("vu1p", BF16)
        Dh = t_("vDh", BF16, w=VHI + 2)
        E0p, E1p = t_("vE0p"), t_("vE1p")
        awd, aw = t_("vawd"), t_("vaw")
        selB = t_("selB", w=3 * 640)
        rcpB = t_("rcpB", BF16, w=3 * 640)
        outO = t_("outO", F32, w=3 * 640)
        DoC = [t_(f"DoC{i}", w=3 * 640) for i in range(2)]
        for t in (u0, u1, u0p, u1p):
            _pad_memsets(nc, t, lo, VHI, FV)
        for t in (E0p, E1p):
            _pad_memsets(nc, t, lo, VHI, FV, eng=g)
        def vw(nm, ci):
            off = 5 * VSEG * ci
            return vin[nm][:, off:off + FV]

        Mw = [vMwB[:, 0:FV], vMwB[:, 5 * VSEG:5 * VSEG + FV]]

        # E clamp deferred from the H phase, applied on the whole V planes
        # in GPSIMD's idle window (gap zeros clamp to e^-5; harmless, the
        # mask gap kills those paths)
        ce = PAD + 10 * VSEG
        g.tensor_scalar_max(vin["vE0"][:, lo:ce], vin["vE0"][:, lo:ce], EM5)
        g.tensor_scalar_max(vin["vE1"][:, lo:ce], vin["vE1"][:, lo:ce], EM5)
        # depth_orig loads for the blend (Pool has slack by now)
        for ci, (c0, cw) in enumerate(VCHUNKS):
            g.dma_start(
                DoC[ci][:, 0:2 * 640].rearrange(
                    "p (s c) -> p s c", s=2)[:, :, 0:cw],
                depth_orig[0, 0:256, c0:c0 + cw].rearrange(
                    "(s p) c -> p s c", p=128))
            g.dma_start(DoC[ci][0:96, 2 * 640:2 * 640 + cw],
                        depth_orig[0, 256:352, c0:c0 + cw])
            g.memset(DoC[ci][96:128, 2 * 640:3 * 640], 0.0)

        pending = None
        for ci, (c0, cw) in enumerate(VCHUNKS):
            m, q = vw("vm", ci), vw("vq", ci)
            D = vw("vD", ci)
            E0, E1 = vw("vE0", ci), vw("vE1", ci)
            _axis_prefix(nc, q, Pp, eN, eP, lo, VHI)
            _axis_main(nc, m, q, E0, E1, D, Dh, u0, u1, Mw[ci], c1, c2,
                       Pp, eN, eP, u0p, u1p, E0p, E1p, lo, VHI,
                       awd[:, lo:VHI], esub_eng=nc.gpsimd)
            _axis_aw_scans(nc, m, E0p, E1p, E0, E1, lo, VHI)
            g.tensor_tensor(aw[:, lo:VHI], E0[:, lo:VHI], E1[:, lo:VHI],
                            op=Alu.add)
            # transpose back through PE with the H+V accumulation fused into
            # the PSUM-read add (in place on twdH/twH)
            _tpose_out_acc(nc, pp, ident, awd, twdH, cw, c0)
            _tpose_out_acc(nc, pp, ident, aw, twH, cw, c0)
            if pending is not None:
                _blend_chunk(nc, bl, DoC[pending[2]], selB, rcpB, outO, *pending[:2])
            pending = (c0, cw, ci)
        # split the final chunk's blend so earlier pieces' stores overlap
        # later pieces' compute (shorter tail)
        c0, cw, ci = pending
        h1 = (cw // 3 + 15) // 16 * 16
        h2 = (2 * cw // 3 + 15) // 16 * 16
        _blend_chunk(nc, bl, DoC[ci], selB, rcpB, outO, c0, h1)
        _blend_chunk(nc, bl, DoC[ci], selB, rcpB, outO, c0 + h1, h2 - h1, off=h1)
        _blend_chunk(nc, bl, DoC[ci], selB, rcpB, outO, c0 + h2, cw - h2, off=h2)


def _blend_chunk(nc, bl, Do, selB, rcpB, outO, c0, cw, off=0):
    """Final blend for image columns [c0, c0+cw) on row-major planes.
    Do/selB/outO are chunk-relative [128, 3, 640] views at offset off."""
    v = nc.vector
    lo = PAD

    def cs(t):
        return t[:, 0:3 * W].rearrange("p (s c) -> p s c", s=3)[:, :, c0:c0 + cw]

    def cr(t):
        return t[:, 0:3 * 640].rearrange("p (s c) -> p s c", s=3)[:, :, off:off + cw]

    mS, twH, twdH = bl["mS"], bl["twH"], bl["twdH"]
    lam_t = bl["lam_t"]
    msk = mS.rearrange("p (s c) -> p s c", s=3)[:, :, lo + c0:lo + c0 + cw]
    nc.gpsimd.tensor_scalar(cr(selB), cs(twH), 0.0, None, op0=Alu.is_gt)
    v.tensor_mul(cr(selB), cr(selB), msk)
    nc.gpsimd.tensor_scalar_max(cs(twH), cs(twH), 1e-6)
    nc.scalar.activation(cr(selB), cr(selB), ActF.Copy, scale=lam_t[:, 0:1])
    # 1/tw via exp(-ln(tw)) on Act (set 6 holds both exp and ln: no swaps)
    nc.scalar.activation(cr(rcpB), cs(twH), ActF.Ln)
    nc.scalar.activation(cr(rcpB), cr(rcpB), ActF.Exp, scale=-1.0)
    v.tensor_mul(cs(twdH), cs(twdH), cr(rcpB))       # lat = twd / tw
    v.tensor_sub(cs(twdH), cs(twdH), cr(Do))         # lat - Do
    v.tensor_mul(cs(twdH), cs(twdH), cr(selB))       # * sel * lam
    v.tensor_tensor(cr(outO), cr(Do), cs(twdH), op=Alu.add)
    for si, (r0, hs) in enumerate(RSEGS):
        rs = slice(r0, r0 + hs)
        nc.sync.dma_start(
            bl["depthout"][0, rs, c0:c0 + cw],
            outO[0:hs, si * 640 + off:si * 640 + off + cw])


_NC = None


def _get_nc():
    global _NC
    if _NC is None:
        _NC = build_program()
    return _NC


def kernel(pred_log, mask, variance, depthin, lam, times):
    pred_log = np.ascontiguousarray(np.asarray(pred_log, dtype=np.float32))
    mask = np.ascontiguousarray(np.asarray(mask, dtype=np.int32))
    variance = np.ascontiguousarray(np.asarray(variance, dtype=np.float32))
    depthin = np.ascontiguousarray(np.asarray(depthin, dtype=np.float32))
    lam = np.ascontiguousarray(np.asarray(lam, dtype=np.float32)).reshape(1)
    t = int(np.asarray(times))

    if t <= 0:
        return depthin.copy()
    nc = _get_nc()
    depth_cur = depthin
    for _ in range(t):
        in_maps = [{
            "pred_log": pred_log[b],
            "mask": mask[b],
            "variance": variance[b],
            "depth_cur": depth_cur[b],
            "depth_orig": depthin[b],
            "lam": lam,
        } for b in range(B)]
        res = run_bass_kernel_spmd(nc, in_maps, list(range(B)))
        depth_cur = np.stack([res.results[i]["depthout"] for i in range(B)])
    return depth_cur.astype(np.float32)


# revision 81
# speedup vs baseline: 1.0202x; 1.0162x over previous
"""CRF integration (nn_CRFIntegrationModule) Trainium2 kernel.

One image per NeuronCore (B=8 -> 8 cores).  Each direction's 32-step windowed
scan is one hardware tensor_tensor_scan on a pre-corrected input (windowed
linear recurrence):

    A32[n] = (A32[n-1] + u'[n-1]) * m[n-1]
    u'[k]  = u[k] - M[k] * u[k -/+ 32]        (M = windowed mask product)

The per-path exponent exp(sum plog) is factored out through the global plog
prefix P (one scan):  awd = e^{P} * scan(u * e^{-P}, m), so every scan
transition is the exact mask and no exp factors compound along paths.
M comes from a mask cumsum scan + windowed difference + ==32 compare, all on
GPSIMD (one engine-local chain).

Data flow: the H phase loads every input plane once (SWDGE cast loads to
fp16, multi-plane fused calls, all enqueued up front in readiness order —
engine queues are in-order FIFOs, so issue order is chosen to match expected
operand arrival) into persistent row-major SBUF planes and runs the
horizontal scans.  The V phase reads its inputs from those SBUF planes
through the XBAR DMA transpose (issued on the otherwise idle SP sequencer:
no compute engine spends time staging or transposing), scans vertically, and
PE-transposes the results back, accumulating into the H totals in place.
Work is split across DVE / GPSIMD / Act so every stream stays busy: the
mask-window machinery and the aw (weight) scans run on GPSIMD, deferred
until their inputs are ready so they never head-block the queue; the final
blend divides on GPSIMD (no Ln/Exp activation-table swaps anywhere).
"""
import os
import sys

for _p in ("/opt/trn_rl_repo", "/root/.axon_site/_ro/trn_rl_repo"):
    if os.path.isdir(_p) and _p not in sys.path:
        sys.path.insert(0, _p)
        break

import numpy as np
import concourse.bacc as bacc
import concourse.mybir as mybir
import concourse.tile as tile
from concourse import masks
from concourse.bass_utils import run_bass_kernel_spmd

Alu = mybir.AluOpType
ActF = mybir.ActivationFunctionType
F32 = mybir.dt.float32
I32 = mybir.dt.int32
BF16 = mybir.dt.bfloat16
F16 = mybir.dt.float16

B, H, W = 8, 352, 1216
R = 32          # MAXRANGE
CLIP = 5.0      # CLIPVARIANCE
EM5 = float(np.exp(-CLIP))
PAD = 32

# H-phase geometry: row segments (partitions = rows), padded row-major planes
RSEGS = [(0, 128), (128, 128), (256, 96)]          # (row0, height)
FH = PAD + 1280                                    # transposes read 1280 wide
HI = PAD + W
FHN = HI + 2                                       # narrow scratch width
# V-phase geometry: transposed layout, 2 chunks x 5 col-segments of <=128 cols
VSEG = H + PAD                                     # 384 per col-seg span
NCS = 5
FV = PAD + NCS * VSEG + 8                          # 1960
VHI = PAD + (NCS - 1) * VSEG + H                   # 1920
FVB = PAD + 2 * NCS * VSEG + 8                     # 3880: both chunks
VCHUNKS = [(0, 640), (640, 576)]


def _pad_memsets(nc, t, lo, hi, F, vgaps=False, eng=None):
    """Zero the pad strips of a [128, F] tile (head, tail, V inter-seg gaps)."""
    g_ = eng if eng is not None else nc.vector
    g_.memset(t[:, 0:lo], 0.0)
    g_.memset(t[:, hi:F], 0.0)
    if vgaps:
        g = t[:, PAD:PAD + 5 * VSEG].rearrange("p (s c) -> p s c", s=5)
        g_.memset(g[:, :, H:VSEG], 0.0)


def _mw_chain(nc, m, Mw, Sm, cs, lo, hi, sub_eng=None):
    """Windowed mask product M[n] = prod_{j=1..32} m[n-j]: cumsum scan (DVE:
    hardware scans only exist there) + windowed difference + ==32 compare."""
    g = nc.gpsimd
    sub = sub_eng if sub_eng is not None else g
    nc.vector.tensor_tensor_scan(
        Sm[:, 1:hi + 33], m[:, 0:hi + 32], m[:, 0:hi + 32],
        0.0, op0=Alu.add, op1=Alu.bypass)
    sub.tensor_tensor(cs[:, lo:hi + 33], Sm[:, lo:hi + 33],
                      Sm[:, lo - R:hi + 33 - R], op=Alu.subtract)
    g.tensor_scalar(Mw[:, lo:hi + 33], cs[:, lo:hi + 33], float(R) - 0.5,
                    None, op0=Alu.is_ge)


def _axis_prefix(nc, p, Pp, eN, eP, lo, hi):
    """Plog prefix scan + its exponentials: first DVE/Act work of a segment
    (issued before the E-plane exp/clamp so it never waits on them)."""
    nc.vector.tensor_tensor_scan(
        Pp[:, lo:hi], p[:, lo - 1:hi - 1], p[:, lo - 1:hi - 1], 0.0,
        op0=Alu.add, op1=Alu.bypass)
    nc.scalar.activation(eN[:, lo:hi], Pp[:, lo:hi], ActF.Exp, scale=-1.0)
    nc.scalar.activation(eP[:, lo:hi], Pp[:, lo:hi], ActF.Exp)


def _axis_main(nc, m, p, E0, E1, D, Dh, u0, u1, Mw, c1, c2,
               Pp, eN, eP, u0p, u1p, E0p, E1p, lo, hi, awd_out,
               esub_eng=None):
    """DVE stream of one axis pair: u-hat, windowed pre-corrections, the two
    awd scans, and the awd output.  The aw (weight) scans BL/BR run on GPSIMD
    and are emitted separately via _axis_aw_scans; prefix via _axis_prefix;
    scan outputs alias u0/u1/E0/E1."""
    v = nc.vector
    sc = v.tensor_tensor_scan
    # u-hat = E * D * e^{-P}
    v.tensor_mul(Dh[:, lo:hi], D[:, lo:hi], eN[:, lo:hi])
    v.tensor_mul(u0[:, lo:hi], E0[:, lo:hi], Dh[:, lo:hi])
    v.tensor_mul(u1[:, lo:hi], E1[:, lo:hi], Dh[:, lo:hi])
    # windowed pre-corrections  u'[m] = u[m] - M * u[m -/+ 32]  (mask-only)
    v.tensor_mul(c1[:, lo:hi], Mw[:, lo:hi], u0[:, lo - R:hi - R])
    v.tensor_sub(u0p[:, lo:hi], u0[:, lo:hi], c1[:, lo:hi])
    v.tensor_mul(c2[:, lo:hi], Mw[:, lo + R + 1:hi + R + 1],
                 u1[:, lo + R:hi + R])
    v.tensor_sub(u1p[:, lo:hi], u1[:, lo:hi], c2[:, lo:hi])
    es = esub_eng if esub_eng is not None else v
    v.tensor_mul(c1[:, lo:hi], Mw[:, lo:hi], E0[:, lo - R:hi - R])
    es.tensor_tensor(E0p[:, lo:hi], E0[:, lo:hi], c1[:, lo:hi],
                     op=Alu.subtract)
    v.tensor_mul(c2[:, lo:hi], Mw[:, lo + R + 1:hi + R + 1],
                 E1[:, lo + R:hi + R])
    es.tensor_tensor(E1p[:, lo:hi], E1[:, lo:hi], c2[:, lo:hi],
                     op=Alu.subtract)
    AL, AR = u0, u1
    sc(AL[:, lo:hi], u0p[:, lo - 1:hi - 1], m[:, lo - 1:hi - 1], 0.0,
       op0=Alu.add, op1=Alu.mult)
    sc(AR[:, lo:hi][:, ::-1], u1p[:, lo + 1:hi + 1][:, ::-1],
       m[:, lo + 1:hi + 1][:, ::-1], 0.0, op0=Alu.add, op1=Alu.mult)
    v.tensor_add(c1[:, lo:hi], AL[:, lo:hi], AR[:, lo:hi])
    v.tensor_mul(awd_out, c1[:, lo:hi], eP[:, lo:hi])


def _axis_aw_scans(nc, m, E0p, E1p, BL, BR, lo, hi, eng=None):
    """Deferred weight scans (aw numerator); DVE (scans are DVE-only)."""
    g = nc.vector
    g.tensor_tensor_scan(BL[:, lo:hi], E0p[:, lo - 1:hi - 1],
                         m[:, lo - 1:hi - 1], 0.0, op0=Alu.add, op1=Alu.mult)
    g.tensor_tensor_scan(BR[:, lo:hi][:, ::-1], E1p[:, lo + 1:hi + 1][:, ::-1],
                         m[:, lo + 1:hi + 1][:, ::-1], 0.0,
                         op0=Alu.add, op1=Alu.mult)


def build_program():
    nc = bacc.Bacc("TRN2", target_bir_lowering=False, debug=False, dynamic_dma_scratch_size=16384)

    pred_log = nc.dram_tensor("pred_log", [2, H, W], F32, kind="ExternalInput").ap()
    mask = nc.dram_tensor("mask", [1, H, W], I32, kind="ExternalInput").ap()
    variance = nc.dram_tensor("variance", [4, H, W], F32, kind="ExternalInput").ap()
    depth_cur = nc.dram_tensor("depth_cur", [1, H, W], F32, kind="ExternalInput").ap()
    depth_orig = nc.dram_tensor("depth_orig", [1, H, W], F32, kind="ExternalInput").ap()
    lam = nc.dram_tensor("lam", [1], F32, kind="ExternalInput").ap()
    depthout = nc.dram_tensor("depthout", [1, H, W], F32, kind="ExternalOutput").ap()

    g = nc.gpsimd
    lo = PAD
    with tile.TileContext(nc, pool_alloc_mode="queue") as tc:
        with tc.tile_pool(name="const", bufs=1) as cp, \
             tc.tile_pool(name="persist", bufs=1) as ps, \
             tc.tile_pool(name="psum", bufs=8, space="PSUM") as pp:
            ident = cp.tile([128, 128], F16, tag="ident")
            masks.make_identity(nc, ident[:])
            # natural_log_exp_and_others: covers every activation we use, so
            # the act-table pass never needs to insert mid-kernel swaps
            nc.scalar.add_instruction(mybir.InstLoadActFuncSet(
                name=nc.get_next_instruction_name(), act_func_set_id=6,
                ins=[], outs=[]))
            # band matrices: the V-direction 32-row window sum is a PE
            # convolution over rows in the row-major layout
            # W1[k, i] = 1 iff i-32 <= k < i (own segment rows)
            W1 = cp.tile([128, 128], F16, tag="W1")
            nc.gpsimd.memset(W1[:, :], 1.0)
            nc.gpsimd.affine_select(W1[:, :], W1[:, :], base=32,
                                    channel_multiplier=1,
                                    pattern=[[-1, 128]],
                                    compare_op=Alu.is_ge, fill=0.0)
            nc.gpsimd.affine_select(W1[:, :], W1[:, :], base=-1,
                                    channel_multiplier=-1,
                                    pattern=[[1, 128]],
                                    compare_op=Alu.is_ge, fill=0.0)
            # W2[k, i] = 1 iff k >= 96 + i (previous segment's tail rows)
            W2 = cp.tile([128, 128], F16, tag="W2")
            nc.gpsimd.memset(W2[:, :], 1.0)
            nc.gpsimd.affine_select(W2[:, :], W2[:, :], base=-96,
                                    channel_multiplier=1,
                                    pattern=[[-1, 128]],
                                    compare_op=Alu.is_ge, fill=0.0)
            lam_t = cp.tile([128, 1], F32, tag="lam")

            # persistent row-major fp16 planes, 3 row-seg slots of width FH
            mS = ps.tile([128, 3 * FH], F16, tag="mS")
            p01S = ps.tile([128, 6 * FH], F16, tag="p01S")    # [seg][p0,p1]
            DS = ps.tile([128, 3 * FH], F16, tag="DS")
            E23S = ps.tile([128, 6 * FH], F16, tag="E23S")    # [seg][E2,E3]
            twH = ps.tile([128, 3 * W], F16, tag="twH")
            twdH = ps.tile([128, 3 * W], F16, tag="twdH")
            # V input planes (written by SP DMA transposes; pads and
            # inter-seg gaps zeroed in the prologue while DVE is idle).
            # One full-width tile per plane: col-groups 0-4 are V chunk 0,
            # 5-9 chunk 1; each chunk's compute uses a [128, FV] window.
            vin = {}
            for nm in ("vm", "vq", "vD", "vE0", "vE1"):
                vin[nm] = ps.tile([128, FVB], F16, tag=f"{nm}B",
                                  name=f"{nm}B")
            MwVH = ps.tile([128, 3 * FH], F16, tag="MwVH")
            vMwB = ps.tile([128, FVB], F16, tag="vMwB")

            # ---- prologue ------------------------------------------------
            # pad zeroing first (everything idle at t=0; loads write only
            # the interiors, but region tracking may be whole-tile, so pads
            # go first to keep the load DMAs unblocked)
            m3 = mS.rearrange("p (s c) -> p s c", s=3)
            p6 = p01S.rearrange("p (s c) -> p s c", s=6)
            D3 = DS.rearrange("p (s c) -> p s c", s=3)
            E6 = E23S.rearrange("p (s c) -> p s c", s=6)
            for t3 in (p6, E6):
                g.memset(t3[:, :, 0:PAD], 0.0)
                g.memset(t3[:, :, HI:FH], 0.0)
            for t3 in (m3, D3):
                nc.vector.memset(t3[:, :, 0:PAD], 0.0)
                nc.vector.memset(t3[:, :, HI:FH], 0.0)
            # seg2 slots hold 96 rows; full-partition ops read rows 96:127
            g.memset(p6[96:128, 4:6, PAD:HI], 0.0)
            nc.vector.memset(m3[96:128, 2:3, PAD:HI], 0.0)
            nc.vector.memset(D3[96:128, 2:3, PAD:HI], 0.0)
            for t in list(vin.values()) + [vMwB]:
                nc.vector.memset(t[:, 0:lo], 0.0)
                nc.vector.memset(t[:, PAD + 10 * VSEG:FVB], 0.0)
                gv = t[:, PAD:PAD + 10 * VSEG].rearrange(
                    "p (s c) -> p s c", s=10)
                nc.vector.memset(gv[:, :, H:VSEG], 0.0)
            mw3 = MwVH.rearrange("p (s c) -> p s c", s=3)
            nc.vector.memset(mw3[:, :, 0:PAD], 0.0)
            nc.vector.memset(mw3[:, :, HI:FH], 0.0)


            _h_phase(nc, tc, mask, pred_log, variance, depth_cur,
                     depth_orig, mS, p01S, DS, E23S, twH, twdH, m3, D3,
                     pp, W1, W2, MwVH)

            # V-input transposes on the idle SP sequencer.  The manual wait
            # overrides keep them out of the early DMA window (the framework
            # serializes the in-flight DMA stream, so transposes scheduled
            # between the critical H loads would push those loads out).
            early, late = _v_transposes(vin, mS, p01S, DS, E23S)
            with tc.tile_wait_until(0.036):
                for args in early:
                    nc.sync.dma_start_transpose(*args)
            with tc.tile_wait_until(0.040):
                for args in late:
                    nc.sync.dma_start_transpose(*args)

            with tc.tile_wait_until(0.040):
                for args in _vmw_transposes(MwVH, vMwB):
                    nc.sync.dma_start_transpose(*args)

            with tc.tile_wait_until(0.040):
                nc.sync.dma_start(lam_t[:, 0:1], lam.partition_broadcast(128))
            bl = dict(mS=mS, twH=twH, twdH=twdH, lam_t=lam_t,
                      depthout=depthout)
            _v_phase(nc, tc, pp, ident, depth_orig, vin, twH, twdH, vMwB, bl)
    nc.finalize()
    return nc


def _h_phase(nc, tc, mask, pred_log, variance, depth, depth_orig,
             mS, p01S, DS, E23S, twH, twdH, m3, D3, pp, W1, W2, MwVH):
    v = nc.vector
    g = nc.gpsimd
    with tc.tile_pool(name="hp", bufs=1) as hp:
        def t_(tag, w=FH, dt=F16, bufs=1):
            return hp.tile([128, w], dt, tag=tag, name=tag, bufs=bufs)

        c1, c2 = t_("c1", dt=BF16), t_("c2", dt=BF16)
        mwc = t_("mwc", dt=BF16)
        u0p, u1p = t_("u0p", dt=BF16), t_("u1p", dt=BF16)
        E0p = [t_(f"E0p{i}") for i in range(3)]
        E1p = [t_(f"E1p{i}") for i in range(3)]
        Mw = [t_(f"Mw{i}") for i in range(3)]
        Sm = t_("Sm")
        E01S = t_("E01S", 6 * FH)                      # [seg][E0,E1]
        for t in (u0p, u1p):
            _pad_memsets(nc, t, PAD, HI, FH)
        e6 = E01S.rearrange("p (s c) -> p s c", s=6)
        nc.vector.memset(e6[:, :, 0:PAD], 0.0)
        nc.vector.memset(e6[:, :, HI:FH], 0.0)
        nc.vector.memset(e6[96:128, 4:6, PAD:HI], 0.0)
        g.memset(Sm[:, 0:1], 0.0)

        # ---- load batch 1: seg0+1 essentials (free DMA-window slots) ----
        pq = p01S.rearrange("p (s c) -> p s c", s=6)
        E6q = E23S.rearrange("p (s c) -> p s c", s=6)
        g.dma_start(pq[:, 0:4:2, PAD:HI],
                    pred_log[0, 0:256, :].rearrange("(s p) c -> p s c", p=128))
        g.dma_start(m3[:, 0:2, PAD:HI],
                    mask[0, 0:256, :].rearrange("(s p) c -> p s c", p=128))
        g.dma_start(e6[:, 0:4:2, PAD:HI],
                    variance[0, 0:256, :].rearrange("(s p) c -> p s c", p=128))
        g.dma_start(e6[:, 1:4:2, PAD:HI],
                    variance[1, 0:256, :].rearrange("(s p) c -> p s c", p=128))
        g.dma_start(D3[:, 0:2, PAD:HI],
                    depth[0, 0:256, :].rearrange("(s p) c -> p s c", p=128))

        # seg0's mask-window chain slots between the two enqueue batches, so
        # the later enqueues' DMA-window waits are already satisfied when the
        # Pool engine reaches them (no head-of-line stall either way)
        _mw_chain(nc, mS[:, 0:FH], Mw[0], Sm, mwc, PAD, HI)

        # ---- load batch 2: seg2 + the V-phase E planes -------------------
        g.dma_start(mS[0:96, 2 * FH + PAD:2 * FH + HI], mask[0, 256:352, :])
        g.dma_start(
            p01S[0:96, 4 * FH:6 * FH].rearrange(
                "p (q c) -> p q c", q=2)[:, :, PAD:HI],
            pred_log[0:2, 256:352, :].rearrange("q r c -> r q c"))
        g.dma_start(DS[0:96, 2 * FH + PAD:2 * FH + HI],
                    depth[0, 256:352, :])
        g.dma_start(
            E01S[0:96, 4 * FH:6 * FH].rearrange(
                "p (q c) -> p q c", q=2)[:, :, PAD:HI],
            variance[0:2, 256:352, :].rearrange("q r c -> r q c"))
        g.dma_start(pq[:, 1:4:2, PAD:HI],
                    pred_log[1, 0:256, :].rearrange("(s p) c -> p s c", p=128))
        g.dma_start(E6q[:, 0:4:2, PAD:HI],
                    variance[2, 0:256, :].rearrange("(s p) c -> p s c", p=128))
        g.dma_start(E6q[:, 1:4:2, PAD:HI],
                    variance[3, 0:256, :].rearrange("(s p) c -> p s c", p=128))
        g.dma_start(
            E23S[0:96, 4 * FH:6 * FH].rearrange(
                "p (q c) -> p q c", q=2)[:, :, PAD:HI],
            variance[2:4, 256:352, :].rearrange("q r c -> r q c"))
        for i in range(3):
            _pad_memsets(nc, E0p[i], PAD, HI, FH, eng=g)
            _pad_memsets(nc, E1p[i], PAD, HI, FH, eng=g)

        segt = []
        for si, (r0, hs) in enumerate(RSEGS):
            m = mS[:, si * FH:(si + 1) * FH]
            p = p01S[:, 2 * si * FH:(2 * si + 1) * FH]
            D = DS[:, si * FH:(si + 1) * FH]
            E23 = E23S[:, 2 * si * FH:(2 * si + 2) * FH]
            E01 = E01S[:, 2 * si * FH:(2 * si + 2) * FH]
            Dh = t_("Dh", w=FHN, dt=BF16, bufs=2)
            u01 = t_("u01", 2 * FH, dt=BF16, bufs=2)
            Pp = t_("Pp", w=FHN, dt=F32, bufs=2)
            eN, eP = (t_("eN", w=FHN, dt=BF16, bufs=2),
                      t_("eP", w=FHN, dt=BF16, bufs=2))
            E0, E1 = E01[:, 0:FH], E01[:, FH:2 * FH]
            u0, u1 = u01[:, 0:FH], u01[:, FH:2 * FH]
            _pad_memsets(nc, u01[:, 0:FH], PAD, HI, FH)
            _pad_memsets(nc, u01[:, FH:2 * FH], PAD, HI, FH)

            _axis_prefix(nc, p, Pp, eN, eP, PAD, HI)
            # E = max(exp(-v), e^-5): in-place exp on Act, clamp on DVE
            e2 = E01[0:hs, 0:2 * FH].rearrange("p (s c) -> p s c", s=2)
            nc.scalar.activation(e2[:, :, PAD:HI], e2[:, :, PAD:HI],
                                 ActF.Exp, scale=-1.0)
            clamp_eng = v if si == 0 else g
            clamp_eng.tensor_scalar_max(
                E01[:, 0:2 * FH].rearrange("p (s c) -> p s c", s=2)[:, :, PAD:HI],
                E01[:, 0:2 * FH].rearrange("p (s c) -> p s c", s=2)[:, :, PAD:HI],
                EM5)
            if si > 0:
                _mw_chain(nc, m, Mw[si], Sm, mwc, PAD, HI)

            _axis_main(nc, m, p, E0, E1, D, Dh, u0, u1, Mw[si], c1, c2,
                       Pp, eN, eP, u0p, u1p, E0p[si], E1p[si], PAD, HI,
                       twdH[:, si * W:(si + 1) * W])

            # E23 for the V phase: exp in place (Act has slack), clamp
            # deferred to the V layout
            e23 = E23[0:hs, 0:2 * FH].rearrange("p (s c) -> p s c", s=2)
            nc.scalar.activation(e23[:, :, PAD:HI], e23[:, :, PAD:HI],
                                 ActF.Exp, scale=-1.0)
            segt.append((m, E0, E1, si))

        # V-direction 32-row window sums on PE (convolution over rows),
        # read out by Act, compared on GPSIMD into the row-major Mw plane
        for si, (r0, hs) in enumerate(RSEGS):
            m_si = mS[:, si * FH:(si + 1) * FH]
            for cc0, ccw in ((0, 512), (512, 512), (1024, 192)):
                pv = pp.tile([128, 512], F32, tag="pv", bufs=2, name="pv")
                if si == 0:
                    nc.tensor.matmul(pv[:, 0:ccw],
                                     W1[0:hs, :],
                                     m_si[0:hs, PAD + cc0:PAD + cc0 + ccw],
                                     start=True, stop=True)
                else:
                    m_pr = mS[:, (si - 1) * FH:si * FH]
                    nc.tensor.matmul(pv[:, 0:ccw],
                                     W1[0:hs, :],
                                     m_si[0:hs, PAD + cc0:PAD + cc0 + ccw],
                                     start=True, stop=False)
                    nc.tensor.matmul(pv[:, 0:ccw],
                                     W2[0:128, :],
                                     m_pr[0:128, PAD + cc0:PAD + cc0 + ccw],
                                     start=False, stop=True)
                nc.scalar.copy(
                    MwVH[:, si * FH + PAD + cc0:si * FH + PAD + cc0 + ccw],
                    pv[:, 0:ccw])
        mw3d = MwVH.rearrange("p (s c) -> p s c", s=3)
        g.tensor_scalar(mw3d[:, :, PAD:HI], mw3d[:, :, PAD:HI],
                        float(R) - 0.5, None, op0=Alu.is_ge)

        # deferred GPSIMD weight scans (need the pre-corrections), then the
        # aw totals on DVE once the scans land
        for m, E0, E1, si in segt:
            _axis_aw_scans(nc, m, E0p[si], E1p[si], E0, E1, PAD, HI)
        for m, E0, E1, si in segt:
            v.tensor_add(twH[:, si * W:(si + 1) * W],
                         E0[:, PAD:HI], E1[:, PAD:HI])



def _v_transposes(vin, mS, p01S, DS, E23S):
    """XBAR transpose call lists (early; late): one call per (plane,
    row-segment) covering the full 1280-col width -> 10 V col-groups.
    seg_base(si) gives the FH-slot offset of row-segment si in the source."""
    early, late = [], []

    def plane_calls(out, src, seg_base, dst):
        o10 = dst[:, PAD:PAD + 10 * VSEG].rearrange("p (s c) -> p s c", s=10)
        for rp, (r0, hs) in enumerate(RSEGS):
            b = seg_base(rp) + PAD
            out.append((o10[:, :, rp * 128:rp * 128 + hs],
                        src[0:hs, b:b + 1280]))

    plane_calls(early, mS, lambda s: s * FH, vin["vm"])
    plane_calls(early, p01S, lambda s: (2 * s + 1) * FH, vin["vq"])
    plane_calls(early, DS, lambda s: s * FH, vin["vD"])
    plane_calls(late, E23S, lambda s: 2 * s * FH, vin["vE0"])
    plane_calls(late, E23S, lambda s: (2 * s + 1) * FH, vin["vE1"])
    return early, late


def _vmw_transposes(MwVH, vMwB):
    """Mw rows include one extra fold for seg2 (rows 96..111): conv row 96
    holds the window ending at the image's last row, which the backward
    pre-correction reads at the first gap position; rows 97+ are real
    zeros (partial windows)."""
    calls = []
    o10 = vMwB[:, PAD:PAD + 10 * VSEG].rearrange("p (s c) -> p s c", s=10)
    for rp, (r0, hs) in enumerate(RSEGS):
        b = rp * FH + PAD
        he = 112 if rp == 2 else hs
        calls.append((o10[:, :, rp * 128:rp * 128 + he],
                      MwVH[0:he, b:b + 1280]))
    return calls


def _tpose_out_acc(nc, pp, ident, src, dst, cw, c0, eng=None):
    """Transposed src [128, FV] fp16 -> row-major: dst += src^T (in place).
    One PSUM-operand tensor_tensor add per merged group."""
    v = eng if eng is not None else nc.vector
    ncs = (cw + 127) // 128
    for rp, (r0, hs) in enumerate(RSEGS):
        cs = 0
        while cs < ncs:
            bw = min(128, cw - cs * 128)
            fb = PAD + cs * VSEG + rp * 128
            ng = 0
            while (cs + ng < ncs and ng < 4
                   and min(128, cw - (cs + ng) * 128) == 128):
                ng += 1
            cb = rp * W + c0 + cs * 128
            if ng >= 2:
                psu = pp.tile([128, 128 * ng], F16, tag="pt2", bufs=3,
                              name="psg")
                for gi in range(ng):
                    nc.tensor.transpose(
                        psu[0:hs, 128 * gi:128 * (gi + 1)],
                        src[:, fb + VSEG * gi:fb + VSEG * gi + hs],
                        ident[:, :])
                v.tensor_tensor(dst[0:hs, cb:cb + 128 * ng],
                                psu[0:hs, 0:128 * ng],
                                dst[0:hs, cb:cb + 128 * ng], op=Alu.add)
                cs += ng
            else:
                psu = pp.tile([128, 128], F16, tag="pt1", bufs=2)
                nc.tensor.transpose(psu[0:hs, 0:bw], src[0:bw, fb:fb + hs],
                                    ident[0:bw, 0:bw])
                v.tensor_tensor(dst[0:hs, cb:cb + bw], psu[0:hs, 0:bw],
                                dst[0:hs, cb:cb + bw], op=Alu.add)
                cs += 1


def _v_phase(nc, tc, pp, ident, depth_orig, vin, twH, twdH, vMwB, bl):
    v = nc.vector
    g = nc.gpsimd
    lo = PAD
    with tc.tile_pool(name="vp", bufs=1) as vp:
        def t_(tag, dt=F16, bufs=1, w=FV):
            return vp.tile([128, w], dt, tag=tag, name=tag, bufs=bufs)

        u0, u1 = t_("vu0", BF16), t_("vu1", BF16)
        c1, c2 = t_("vc1", BF16), t_("vc2", BF16)
        Pp = t_("vPp", F32, w=VHI + 2)
        eN, eP = t_("veN", BF16, w=VHI + 2), t_("veP", BF16, w=VHI + 2)
        u0p, u1p = t_("vu0p", BF16), t_("vu1p", BF16)
        Dh = t_("vDh", BF16, w=VHI + 2)
        E0p, E1p = t_("vE0p"), t_("vE1p")
        awd, aw = t_("vawd"), t_("vaw")
        selB = t_("selB", w=3 * 640)
        rcpB = t_("rcpB", BF16, w=3 * 640)
        outO = t_("outO", F32, w=3 * 640)
        DoC = [t_(f"DoC{i}", w=3 * 640) for i in range(2)]
        for t in (u0, u1, u0p, u1p):
            _pad_memsets(nc, t, lo, VHI, FV)
        for t in (E0p, E1p):
            _pad_memsets(nc, t, lo, VHI, FV, eng=g)
        def vw(nm, ci):
            off = 5 * VSEG * ci
            return vin[nm][:, off:off + FV]

        Mw = [vMwB[:, 0:FV], vMwB[:, 5 * VSEG:5 * VSEG + FV]]

        # E clamp deferred from the H phase, applied on the whole V planes
        # in GPSIMD's idle window (gap zeros clamp to e^-5; harmless, the
        # mask gap kills those paths)
        ce = PAD + 10 * VSEG
        g.tensor_scalar_max(vin["vE0"][:, lo:ce], vin["vE0"][:, lo:ce], EM5)
        g.tensor_scalar_max(vin["vE1"][:, lo:ce], vin["vE1"][:, lo:ce], EM5)
        # depth_orig loads for the blend (Pool has slack by now)
        for ci, (c0, cw) in enumerate(VCHUNKS):
            g.dma_start(
                DoC[ci][:, 0:2 * 640].rearrange(
                    "p (s c) -> p s c", s=2)[:, :, 0:cw],
                depth_orig[0, 0:256, c0:c0 + cw].rearrange(
                    "(s p) c -> p s c", p=128))
            g.dma_start(DoC[ci][0:96, 2 * 640:2 * 640 + cw],
                        depth_orig[0, 256:352, c0:c0 + cw])
            g.memset(DoC[ci][96:128, 2 * 640:3 * 640], 0.0)

        pending = None
        for ci, (c0, cw) in enumerate(VCHUNKS):
            m, q = vw("vm", ci), vw("vq", ci)
            D = vw("vD", ci)
            E0, E1 = vw("vE0", ci), vw("vE1", ci)
            _axis_prefix(nc, q, Pp, eN, eP, lo, VHI)
            _axis_main(nc, m, q, E0, E1, D, Dh, u0, u1, Mw[ci], c1, c2,
                       Pp, eN, eP, u0p, u1p, E0p, E1p, lo, VHI,
                       awd[:, lo:VHI], esub_eng=nc.gpsimd)
            _axis_aw_scans(nc, m, E0p, E1p, E0, E1, lo, VHI)
            g.tensor_tensor(aw[:, lo:VHI], E0[:, lo:VHI], E1[:, lo:VHI],
                            op=Alu.add)
            # transpose back through PE with the H+V accumulation fused into
            # the PSUM-read add (in place on twdH/twH)
            _tpose_out_acc(nc, pp, ident, awd, twdH, cw, c0)
            _tpose_out_acc(nc, pp, ident, aw, twH, cw, c0)
            if pending is not None:
                pc0, pcw, pci = pending
                ph = (pcw // 2 + 15) // 16 * 16
                _blend_chunk(nc, bl, DoC[pci], selB, rcpB, outO, pc0, ph)
                _blend_chunk(nc, bl, DoC[pci], selB, rcpB, outO,
                             pc0 + ph, pcw - ph, off=ph)
            pending = (c0, cw, ci)
        # split the final chunk's blend so earlier pieces' stores overlap
        # later pieces' compute (shorter tail)
        c0, cw, ci = pending
        h1 = (cw // 3 + 15) // 16 * 16
        h2 = (2 * cw // 3 + 15) // 16 * 16
        _blend_chunk(nc, bl, DoC[ci], selB, rcpB, outO, c0, h1)
        _blend_chunk(nc, bl, DoC[ci], selB, rcpB, outO, c0 + h1, h2 - h1, off=h1)
        _blend_chunk(nc, bl, DoC[ci], selB, rcpB, outO, c0 + h2, cw - h2, off=h2)


def _blend_chunk(nc, bl, Do, selB, rcpB, outO, c0, cw, off=0):
    """Final blend for image columns [c0, c0+cw) on row-major planes.
    Do/selB/outO are chunk-relative [128, 3, 640] views at offset off."""
    v = nc.vector
    lo = PAD

    def cs(t):
        return t[:, 0:3 * W].rearrange("p (s c) -> p s c", s=3)[:, :, c0:c0 + cw]

    def cr(t):
        return t[:, 0:3 * 640].rearrange("p (s c) -> p s c", s=3)[:, :, off:off + cw]

    mS, twH, twdH = bl["mS"], bl["twH"], bl["twdH"]
    lam_t = bl["lam_t"]
    msk = mS.rearrange("p (s c) -> p s c", s=3)[:, :, lo + c0:lo + c0 + cw]
    nc.gpsimd.tensor_scalar(cr(selB), cs(twH), 0.0, None, op0=Alu.is_gt)
    v.tensor_mul(cr(selB), cr(selB), msk)
    nc.gpsimd.tensor_scalar_max(cs(twH), cs(twH), 1e-6)
    nc.scalar.activation(cr(selB), cr(selB), ActF.Copy, scale=lam_t[:, 0:1])
    # 1/tw via exp(-ln(tw)) on Act (set 6 holds both exp and ln: no swaps)
    nc.scalar.activation(cr(rcpB), cs(twH), ActF.Ln)
    nc.scalar.activation(cr(rcpB), cr(rcpB), ActF.Exp, scale=-1.0)
    v.tensor_mul(cs(twdH), cs(twdH), cr(rcpB))       # lat = twd / tw
    v.tensor_sub(cs(twdH), cs(twdH), cr(Do))         # lat - Do
    v.tensor_mul(cs(twdH), cs(twdH), cr(selB))       # * sel * lam
    v.tensor_tensor(cr(outO), cr(Do), cs(twdH), op=Alu.add)
    for si, (r0, hs) in enumerate(RSEGS):
        rs = slice(r0, r0 + hs)
        nc.sync.dma_start(
            bl["depthout"][0, rs, c0:c0 + cw],
            outO[0:hs, si * 640 + off:si * 640 + off + cw])


_NC = None


def _get_nc():
    global _NC
    if _NC is None:
        _NC = build_program()
    return _NC


def kernel(pred_log, mask, variance, depthin, lam, times):
    pred_log = np.ascontiguousarray(np.asarray(pred_log, dtype=np.float32))
    mask = np.ascontiguousarray(np.asarray(mask, dtype=np.int32))
    variance = np.ascontiguousarray(np.asarray(variance, dtype=np.float32))
    depthin = np.ascontiguousarray(np.asarray(depthin, dtype=np.float32))
    lam = np.ascontiguousarray(np.asarray(lam, dtype=np.float32)).reshape(1)
    t = int(np.asarray(times))

    if t <= 0:
        return depthin.copy()
    nc = _get_nc()
    depth_cur = depthin
    for _ in range(t):
        in_maps = [{
            "pred_log": pred_log[b],
            "mask": mask[b],
            "variance": variance[b],
            "depth_cur": depth_cur[b],
            "depth_orig": depthin[b],
            "lam": lam,
        } for b in range(B)]
        res = run_bass_kernel_spmd(nc, in_maps, list(range(B)))
        depth_cur = np.stack([res.results[i]["depthout"] for i in range(B)])
    return depth_cur.astype(np.float32)
